# revision 15
# baseline (speedup 1.0000x reference)
"""GATv2 graph layer Bass kernel for TRN2 (SPMD across 8 NeuronCores, no
collectives).

Design (v3): edges sorted by destination node and sharded across cores by dst
range. Each core builds ONE fp16 gather table in DRAM (xsrc = node_emb @
W_src, split in two halves so int16 gather indices fit), with rows remapped so
table writes use 1KB descriptors. Destination-side features are NOT gathered
from DRAM: x_dst for the core's own nodes lives in SBUF ([P, nw, HID] f16),
and per edge chunk (128 edges, one 128-node dst window) the dst contribution
is gathered by the TensorEngine via a transposed one-hot:

  oh[e, d]  = (dstr[e] == d)                       (DVE tensor_scalar)
  ohT[d, e] = transpose(oh)                        (PE transpose, f16 PSUM)
  psum_combT[hid, e] = xdst_win^T @ ohT            (lhsT=xdst_win, run-batched)
                     + emb8^T @ ohetT              (one matmul per group)
                     + xs_chunk^T                  (lhsT=xs, rhs=identity)
  combT16 = Prelu(psum_combT, alpha=0.2)           (Act)
  ex_ps[e, h] = combT16^T @ att_blk                (PE)
  ex16 = Exp(ex_ps)                                (Act, per group)
  wgt = xs * bcast(ex16)                           (DVE, 4 chunks per op)
  win_ps[dst, 4+HID] += oh^T @ [ex16 | wgt]        (PE scatter)

The hidden dim is stored d-major ((d,h) instead of (h,d), a host-side column
permutation of W_src/W_dst/emb/att/W_out) so the per-head broadcast of ex in
the weight-mul has a packed innermost dim — that enables the DVE 2x perf mode
and 4-chunk batching. FiLM (tanh) and the W_out column scaling are folded
into constants on the host, so the only device activation functions are
{Prelu, Exp, Ln} — all within one activation table (no reload thrash; the
LayerNorm rstd is exp(-0.5*ln(var+eps))).

Window flush (batched across the slab's windows): agg = sum(ex*xs)/sum(ex),
@Wosc + beta_eff, residual, LayerNorm via bn_stats/bn_aggr, f16 DMA out.
No max-subtraction in the softmax: logits are bounded so exp stays finite in
f32 (matches reference to ~1e-4).
"""
import numpy as np
from contextlib import ExitStack
from dataclasses import dataclass

import concourse.bass as bass
import concourse.tile as tile
from concourse import bacc, mybir
from concourse.masks import make_identity

P = 128
HID = 128
H = 4
HD = 32
NET = 8
EPS_LN = 1e-5
MAXCALL = 4096  # max idxs per dma_gather call
DEAD = -5.0     # dst_rel for padding slots (matches no one-hot column)


@dataclass
class Geo:
    N: int
    n_cores: int
    slab_w: int = 3     # windows per slab
    lookahead: int = 2  # slabs of input prefetch
    ohT_act_mod: int = 3    # every k-th group's ohT copy runs on Act
    oh_pool_mod: int = 4    # every k-th chunk's oh gen runs on Pool
    wmul_pool_mod: int = 3  # every k-th chunk's weight-mul runs on Pool

    @property
    def npc(self):
        return self.N // self.n_cores

    @property
    def nw(self):
        return (self.npc + P - 1) // P

    @property
    def nslab(self):
        return (self.nw + self.slab_w - 1) // self.slab_w

    @property
    def n_pad(self):   # padded node count (512 blocks)
        return ((self.N + 511) // 512) * 512

    @property
    def split(self):   # lo/hi table split on a 512 block boundary
        return (self.n_pad // 1024) * 512


def wrap_idx(idx, cols):
    n = idx.shape[0]
    assert n % 16 == 0
    w = np.zeros((P, cols), dtype=np.int16)
    if n:
        t16 = idx.reshape(n // 16, 16).T
        for g in range(8):
            w[g * 16:(g + 1) * 16, :n // 16] = t16
    return w


def remap_row(r):
    """Table row remap so device-side table writes are 1KB/partition:
    original row i*512 + s*128 + p is stored at i*512 + p*4 + s."""
    i, rem = r // 512, r % 512
    return i * 512 + (rem % 128) * 4 + rem // 128


def host_prep(g: Geo, node_embeddings, edge_index, edge_type, task_embedding,
              W_src, b_src, W_dst, b_dst, edge_emb, att,
              W_out, b_out, norm_w, norm_b, W_film, b_film):
    """Returns (sched, in_maps). Pure index work + tiny constant folding;
    all O(N*HID) / O(E*HID) float math runs on device."""
    src = np.asarray(edge_index[0], dtype=np.int64)
    dst = np.asarray(edge_index[1], dtype=np.int64)
    et = np.asarray(edge_type, dtype=np.int64)
    npc, split = g.npc, g.split

    order = np.argsort(dst, kind="stable")
    src, dst, et = src[order], dst[order], et[order]
    core_of = dst // npc

    buckets = {}
    for c in range(g.n_cores):
        m = core_of == c
        cs, cd, ce = src[m], dst[m] - c * npc, et[m]
        for w in range(g.nw):
            wm = (cd // P) == w
            ws_, wd, we = cs[wm], cd[wm] - w * P, ce[wm]
            lo = ws_ < split
            buckets[(c, w, 0)] = (remap_row(ws_[lo]), wd[lo], we[lo])
            buckets[(c, w, 1)] = (remap_row(ws_[~lo] - split), wd[~lo], we[~lo])

    caps = np.zeros((g.nw, 2), dtype=np.int64)
    for w in range(g.nw):
        for h in range(2):
            mx = max(len(buckets[(c, w, h)][0]) for c in range(g.n_cores))
            caps[w, h] = (mx + P - 1) // P

    # ---- schedule ---------------------------------------------------------
    sched_slabs = []
    total_chunks = 0
    for s in range(g.nslab):
        ws = list(range(s * g.slab_w, min((s + 1) * g.slab_w, g.nw)))
        chunks = []            # (win_local, half, slot)
        calls = {0: [], 1: []}
        slot = 0
        for h in (0, 1):
            run = 0
            run_start = slot
            for w in ws:
                for _ in range(caps[w, h]):
                    chunks.append((w - ws[0], h, slot))
                    slot += 1
                    run += P
                    if run == MAXCALL:
                        calls[h].append((run_start, run))
                        run, run_start = 0, slot
            if run:
                calls[h].append((run_start, run))
        sched_slabs.append(dict(windows=ws, chunks=chunks, calls=calls,
                                chunk0=total_chunks))
        total_chunks += len(chunks)

    lo_cols = max(16, sum(n for sl in sched_slabs
                          for (_, n) in sl["calls"][0]) // 16)
    hi_cols = max(16, sum(n for sl in sched_slabs
                          for (_, n) in sl["calls"][1]) // 16)

    # ---- shared constants -------------------------------------------------
    # d-major permutation of the hidden dim: new col j holds old col
    # (j%H)*HD + j//H, i.e. feature (h, d) lives at j = d*H + h.
    perm = np.array([(j % H) * HD + j // H for j in range(HID)], dtype=np.int64)

    nodeT = np.zeros((HID, g.n_pad), dtype=np.float16)
    nodeT[:, :g.N] = np.asarray(node_embeddings, np.float32).T.astype(np.float16)
    emb_eff = (np.asarray(edge_emb, np.float64)
               + np.asarray(b_src, np.float64)[None, :]
               + np.asarray(b_dst, np.float64)[None, :])[:, perm].astype(np.float16)
    att_blk = np.zeros((HID, H), dtype=np.float16)
    att64 = np.asarray(att, np.float64)
    for h in range(H):
        for d in range(HD):
            att_blk[d * H + h, h] = att64[h, d]

    # FiLM fold (O(HID^2) host math): gamma/beta modulation of the output
    # projection becomes a column scale on W_out plus a bias.
    film = (np.asarray(task_embedding, np.float64)
            @ np.asarray(W_film, np.float64)
            + np.asarray(b_film, np.float64))
    gm = 1.0 + 0.5 * np.tanh(film[:HID])
    beta_eff = np.asarray(b_out, np.float64) * gm + film[HID:]
    Wosc = (np.asarray(W_out, np.float64)[perm, :] * gm[None, :])

    consts = dict(
        nodeT=nodeT,
        W_src=np.asarray(W_src, np.float32)[:, perm].astype(np.float16),
        W_dst=np.asarray(W_dst, np.float32)[:, perm].astype(np.float16),
        Wosc=Wosc.astype(np.float16),
        beta=beta_eff.astype(np.float16).reshape(1, HID),
        emb8=emb_eff,                       # [8, HID] f16 (permuted)
        att_blk=att_blk,
    )
    skip_norm = bool(np.all(np.asarray(norm_w) == 1.0)
                     and np.all(np.asarray(norm_b) == 0.0))
    if not skip_norm:
        consts["normw"] = np.asarray(norm_w, np.float32).reshape(1, HID)
        consts["normb"] = np.asarray(norm_b, np.float32).reshape(1, HID)

    node_f16 = np.asarray(node_embeddings, np.float32).astype(np.float16)

    # ---- per-core arrays --------------------------------------------------
    in_maps = []
    for c in range(g.n_cores):
        lo_l, hi_l = [], []
        dstr = np.full((P, total_chunks), DEAD, dtype=np.float32)
        oet = np.zeros((NET, total_chunks * P), dtype=np.float16)
        ci = 0
        for sl in sched_slabs:
            ws0 = sl["windows"][0]
            per_half = {0: [], 1: []}
            nth = {}
            for (wl, h, slot) in sl["chunks"]:
                w = ws0 + wl
                es, ed, ee = buckets[(c, w, h)]
                k = nth.get((wl, h), 0)
                nth[(wl, h)] = k + 1
                sl_src = np.zeros(P, dtype=np.int64)
                n = min(P, max(0, len(es) - k * P))
                if n > 0:
                    sl_src[:n] = es[k * P:k * P + n]
                    dstr[:n, ci] = ed[k * P:k * P + n]
                    oet[ee[k * P:k * P + n], ci * P + np.arange(n)] = 1.0
                per_half[h].append(sl_src)
                ci += 1
            lo_l.extend(per_half[0])
            hi_l.extend(per_half[1])
        lo_i = (np.concatenate(lo_l) if lo_l else np.zeros(0, np.int64))
        hi_i = (np.concatenate(hi_l) if hi_l else np.zeros(0, np.int64))
        assert lo_i.max(initial=0) < g.split <= 32767
        assert hi_i.max(initial=0) < g.n_pad - g.split <= 32768

        own = node_f16[c * npc:(c + 1) * npc]          # [npc, HID] f16
        ownT = np.zeros((HID, g.nw * P), dtype=np.float16)
        ownT[:, :npc] = own.T

        m = dict(consts)
        m["node_own16"] = np.ascontiguousarray(own)
        m["node_ownT"] = ownT
        m["lo_idx"] = wrap_idx(lo_i.astype(np.int16), lo_cols)
        m["hi_idx"] = wrap_idx(hi_i.astype(np.int16), hi_cols)
        m["dstr"] = dstr
        m["ohetT"] = oet
        in_maps.append(m)

    sched = dict(slabs=sched_slabs, caps=caps, total_chunks=total_chunks,
                 lo_cols=lo_cols, hi_cols=hi_cols, skip_norm=skip_norm)
    return sched, in_maps


def build_program(g: Geo, sched, debug=False):
    nc = bacc.Bacc("TRN2", target_bir_lowering=False, debug=False,
                   num_devices=g.n_cores, num_swdge_queues=4)
    f16, f32 = mybir.dt.float16, mybir.dt.float32
    AF = mybir.ActivationFunctionType
    OP = mybir.AluOpType
    npc, nw = g.npc, g.nw
    total_chunks = sched["total_chunks"]
    lo_cols, hi_cols = sched["lo_cols"], sched["hi_cols"]

    def din(name, shape, dt):
        return nc.dram_tensor(name, shape, dt, kind="ExternalInput").ap()

    nodeT = din("nodeT", [HID, g.n_pad], f16)
    node_own16 = din("node_own16", [npc, HID], f16)
    node_ownT = din("node_ownT", [HID, nw * P], f16)
    W_src = din("W_src", [HID, HID], f16)
    W_dst = din("W_dst", [HID, HID], f16)
    Wosc_dr = din("Wosc", [HID, HID], f16)
    beta_dr = din("beta", [1, HID], f16)
    emb8 = din("emb8", [NET, HID], f16)
    att_blk = din("att_blk", [HID, H], f16)
    lo_idx = din("lo_idx", [P, lo_cols], mybir.dt.int16)
    hi_idx = din("hi_idx", [P, hi_cols], mybir.dt.int16)
    dstr = din("dstr", [P, total_chunks], f32)
    ohetT = din("ohetT", [NET, total_chunks * P], f16)
    out = nc.dram_tensor("out", [npc, HID], f16, kind="ExternalOutput").ap()

    xsrc_tab = nc.dram_tensor("xsrc_tab", [g.n_pad, HID], f16,
                              kind="Internal").ap()

    with tile.TileContext(nc, trace_sim=False) as tc, ExitStack() as ctx:
        cpool = ctx.enter_context(tc.tile_pool(name="consts", bufs=1))
        bpool = ctx.enter_context(tc.tile_pool(name="build", bufs=3))
        # PSUM: 8 banks of 2KB/partition, one per tile buf. Exactly 8 bufs.
        psBig = ctx.enter_context(tc.tile_pool(name="psBig", bufs=2,
                                               space="PSUM"))
        psT4 = ctx.enter_context(tc.tile_pool(name="psT4", bufs=2,
                                              space="PSUM"))
        psWin = ctx.enter_context(tc.tile_pool(name="psWin", bufs=2,
                                               space="PSUM"))
        psEx = ctx.enter_context(tc.tile_pool(name="psEx", bufs=1,
                                              space="PSUM"))
        psSm = ctx.enter_context(tc.tile_pool(name="psSm", bufs=1,
                                              space="PSUM"))
        spool = ctx.enter_context(tc.tile_pool(name="slab",
                                               bufs=g.lookahead + 1))
        gpool = ctx.enter_context(tc.tile_pool(name="grp", bufs=3))
        fpool = ctx.enter_context(tc.tile_pool(name="flush", bufs=2))

        # ---- idx + dstr staging (first: gathers only wait on the table) ---
        lo_sb = cpool.tile([P, lo_cols], mybir.dt.int16, tag="loidx")
        nc.sync.dma_start(lo_sb[:], lo_idx[:])
        hi_sb = cpool.tile([P, hi_cols], mybir.dt.int16, tag="hiidx")
        nc.sync.dma_start(hi_sb[:], hi_idx[:])
        dstr_sb = cpool.tile([P, total_chunks], f32, tag="dstr")
        nc.sync.dma_start(dstr_sb[:], dstr[:])

        # ---- constants ----------------------------------------------------
        ident = cpool.tile([P, P], f16)
        make_identity(nc, ident[:])
        iota16 = cpool.tile([P, P], mybir.dt.int16)
        nc.gpsimd.iota(iota16[:], pattern=[[1, P]], base=0, channel_multiplier=0)
        iota = cpool.tile([P, P], f16)
        nc.vector.tensor_copy(iota[:], iota16[:])
        ones_row = cpool.tile([1, P], f16)
        nc.vector.memset(ones_row[:], 1.0)
        eps_col = cpool.tile([P, 1], f32)
        nc.vector.memset(eps_col[:], EPS_LN)

        Ws = cpool.tile([HID, HID], f16)
        nc.sync.dma_start(Ws[:], W_src[:])
        Wd = cpool.tile([HID, HID], f16)
        nc.sync.dma_start(Wd[:], W_dst[:])
        Wosc = cpool.tile([HID, HID], f16)
        nc.sync.dma_start(Wosc[:], Wosc_dr[:])
        beta16 = cpool.tile([1, HID], f16)
        nc.sync.dma_start(beta16[:], beta_dr[:])
        emb_sb = cpool.tile([NET, HID], f16)
        nc.sync.dma_start(emb_sb[:], emb8[:])
        att_sb = cpool.tile([HID, H], f16)
        nc.sync.dma_start(att_sb[:], att_blk[:])

        # residual (own node embeddings) as [P, nw, HID] f16
        resid16 = cpool.tile([P, nw, HID], f16, tag="resid")
        tail = npc - (npc // P) * P
        full_w = npc // P
        if tail:
            nc.vector.memset(resid16[:, full_w, :], 0.0)
        if full_w:
            nc.sync.dma_start(
                resid16[:, :full_w, :],
                node_own16[:full_w * P, :].rearrange("(w p) h -> p w h", p=P))
        if tail:
            nc.sync.dma_start(resid16[:tail, full_w, :],
                              node_own16[full_w * P:, :])

        if not sched["skip_norm"]:
            nw_dr = din("normw", [1, HID], f32)
            nb_dr = din("normb", [1, HID], f32)
            nw_sb = cpool.tile([1, HID], f32)
            nc.sync.dma_start(nw_sb[:], nw_dr[:])
            nb_sb = cpool.tile([1, HID], f32)
            nc.sync.dma_start(nb_sb[:], nb_dr[:])
            ones32 = cpool.tile([1, P], f32)
            nc.vector.memset(ones32[:], 1.0)
            ps_w = psBig.tile([P, HID], f32, space="PSUM", tag="pcb")
            nc.tensor.matmul(out=ps_w[:], lhsT=ones32[:], rhs=nw_sb[:],
                             start=True, stop=True)
            w_rep = cpool.tile([P, HID], f32)
            nc.vector.tensor_copy(w_rep[:], ps_w[:])
            ps_b = psBig.tile([P, HID], f32, space="PSUM", tag="pcb")
            nc.tensor.matmul(out=ps_b[:], lhsT=ones32[:], rhs=nb_sb[:],
                             start=True, stop=True)
            b_rep = cpool.tile([P, HID], f32)
            nc.vector.tensor_copy(b_rep[:], ps_b[:])

        # ---- xdst table (SBUF-resident) -----------------------------------
        xdst_sb = cpool.tile([P, nw, HID], f16, tag="xdst")
        for gl in range(0, nw, 16):
            gln = min(16, nw - gl)
            not_t = bpool.tile([HID, 16 * P], f16, tag="not")
            nc.sync.dma_start(not_t[:, :gln * P],
                              node_ownT[:, gl * P:(gl + gln) * P])
            for gw in range(gl, gl + gln, 4):
                gn = min(4, gl + gln - gw)
                psX = psBig.tile([P, 4, HID], f32, space="PSUM", tag="pcb")
                for k in range(gn):
                    nc.tensor.matmul(out=psX[:, k, :],
                                     lhsT=not_t[:, (gw - gl + k) * P:
                                                (gw - gl + k + 1) * P],
                                     rhs=Wd[:],
                                     start=True, stop=True,
                                     skip_group_check=True)
                nc.scalar.activation(xdst_sb[:, gw:gw + gn, :], psX[:, :gn, :],
                                     AF.Prelu, alpha=1.0)

        # ---- xsrc gather table --------------------------------------------
        # 2048-node iterations: one big nodeT load (SP) + one big table write
        # (Act) each -- HWDGE charges ~0.6us per DMA instruction, so few big
        # DMAs beat many small ones. PSUM->SBUF copies alternate DVE/Act.
        starts = list(range(0, g.n_pad, 2048))  # last block may be partial
        nts = []
        for i, st in enumerate(starts):
            wdt = min(2048, g.n_pad - st)
            nt = bpool.tile([HID, 2048], f16, tag="nt", name=f"nt{i}",
                            bufs=3)
            nc.sync.dma_start(nt[:, :wdt], nodeT[:, st:st + wdt])
            nts.append(nt)
        for i, st in enumerate(starts):
            wdt = min(2048, g.n_pad - st)
            nq = wdt // 512
            nt = nts[i]
            xt = bpool.tile([P, 4, 4, HID], f16, tag="xt", bufs=3)
            for q in range(nq):
                ps = psBig.tile([P, 4, HID], f32, space="PSUM", tag="pcb")
                for j in range(4):
                    nc.tensor.matmul(out=ps[:, j, :],
                                     lhsT=nt[:, q * 512 + j * P:
                                             q * 512 + (j + 1) * P],
                                     rhs=Ws[:],
                                     start=True, stop=True,
                                     skip_group_check=True)
                if q % 2 == 0:
                    nc.vector.tensor_copy(xt[:, q, :, :], ps[:])
                else:
                    nc.scalar.activation(xt[:, q, :, :], ps[:],
                                         AF.Prelu, alpha=1.0)
            # dram row st + q*512 + p*4 + s  <-  xt[p, q, s, :]
            nc.scalar.dma_start(
                xsrc_tab[st:st + wdt, :]
                .rearrange("(q p s) h -> p q s h", p=P, s=4),
                xt[:, :nq, :, :])

        off16 = {0: 0, 1: 0}
        qn = {0: 0, 1: 0}

        def nextq(h):
            # dedicated queues per table half: a hi gather parked on the full
            # table must not block later lo gathers (lo half is ready first)
            qn[h] ^= 1
            return 2 * h + qn[h]

        # ---- edge slabs ----------------------------------------------------
        # Software-pipelined emission: each slab's input loads and one-hot
        # generation ("front") are emitted `lookahead` slabs ahead of its
        # compute + flush ("back"), so the in-order per-engine sequencers
        # never park a next-slab load behind a previous slab's dependency
        # waits.
        fronts = {}
        wins = {}

        def emit_front(s):
            sl = sched["slabs"][s]
            chunks = sl["chunks"]
            C = len(chunks)
            c0 = sl["chunk0"]

            xs_t = spool.tile([P, C, HID], f16, tag="xs", name=f"xs{s}")
            for h in (0, 1):
                base = 0 if h == 0 else g.split
                lim = g.split if h == 0 else g.n_pad
                idx_sb = lo_sb if h == 0 else hi_sb
                for (slot_off, n) in sl["calls"][h]:
                    if n == 0:
                        continue
                    nc.gpsimd.dma_gather(
                        out_ap=xs_t[:, slot_off:slot_off + n // P, :],
                        in_ap=xsrc_tab[base:lim, :],
                        idxs_ap=idx_sb[:, off16[h]:off16[h] + n // 16],
                        num_idxs=n, num_idxs_reg=n, elem_size=HID,
                        single_packet=(n <= 1024), queue_num=nextq(h),
                    )
                    off16[h] += n // 16

            oet_t = spool.tile([NET, C * P], f16, tag="oet", name=f"oet{s}")
            nc.sync.dma_start(oet_t[:], ohetT[:, c0 * P:(c0 + C) * P])

            oh_t = spool.tile([P, C, P], f16, tag="oh", name=f"oh{s}")
            for ci in range(C):
                eng = (nc.gpsimd if ci % g.oh_pool_mod == g.oh_pool_mod - 1
                       else nc.vector)
                eng.tensor_scalar(
                    oh_t[:, ci, :], iota[:], dstr_sb[:, c0 + ci:c0 + ci + 1],
                    None, OP.is_equal)
            fronts[s] = (xs_t, oet_t, oh_t)

        def emit_back(s):
            sl = sched["slabs"][s]
            ws = sl["windows"]
            nwin = len(ws)
            chunks = sl["chunks"]
            C = len(chunks)
            c0 = sl["chunk0"]
            xs_t, oet_t, oh_t = fronts.pop(s)

            ohT_t = spool.tile([P, C, P], f16, tag="ohT", name=f"ohT{s}",
                               bufs=2)
            rhs_t = spool.tile([P, C, 4 + HID], f16, tag="rhs",
                               name=f"rhs{s}", bufs=2)
            ex_ps = psEx.tile([P, C, H], f32, space="PSUM", tag="ex")

            # Group-level software pipeline (stagger 1): the PE sequencer is
            # in-order and the cost model's DVFS ramp punishes every PE stall,
            # so each PE stage must only consume results produced >= 1 group
            # earlier on the other engines.
            ngrp = (C + 3) // 4

            def g_n(gi):
                return min(4, C - 4 * gi)

            pcbs = {}

            def stage_transpose(gi):          # PE <- oh (front, ready)
                gn, cs0 = g_n(gi), 4 * gi
                psT = psT4.tile([P, 4, P], f16, space="PSUM", tag="psT",
                                name=f"psT{s}_{gi}")
                for k in range(gn):
                    nc.tensor.transpose(out=psT[:, k, :],
                                        in_=oh_t[:, cs0 + k, :],
                                        identity=ident[:])
                return psT

            def stage_copy(gi, psT):          # DVE/Act <- psT
                gn, cs0 = g_n(gi), 4 * gi
                if gi % g.ohT_act_mod == g.ohT_act_mod - 1:
                    nc.scalar.activation(ohT_t[:, cs0:cs0 + gn, :],
                                         psT[:, :gn, :], AF.Prelu, alpha=1.0)
                else:
                    nc.vector.tensor_copy(ohT_t[:, cs0:cs0 + gn, :],
                                          psT[:, :gn, :])

            def stage_pcb(gi):                # PE <- ohT copy (1 group ago)
                gn, cs0 = g_n(gi), 4 * gi
                pcb = psBig.tile([HID, 4, P], f32, space="PSUM", tag="pcb",
                                 name=f"pcb{s}_{gi}")
                for k in range(gn):
                    ci = cs0 + k
                    wl = chunks[ci][0]
                    nc.tensor.matmul(out=pcb[:, k, :],
                                     lhsT=xdst_sb[:, ws[wl], :],
                                     rhs=ohT_t[:, ci, :],
                                     start=True, stop=False,
                                     skip_group_check=True)
                    nc.tensor.matmul(out=pcb[:, k, :], lhsT=emb_sb[:],
                                     rhs=oet_t[:, ci * P:(ci + 1) * P],
                                     start=False, stop=False,
                                     skip_group_check=True)
                    nc.tensor.matmul(out=pcb[:, k, :], lhsT=xs_t[:, ci, :],
                                     rhs=ident[:], start=False, stop=True,
                                     skip_group_check=True)
                pcbs[gi] = pcb

            def stage_prelu(gi):              # Act <- pcb
                gn, cs0 = g_n(gi), 4 * gi
                combT = gpool.tile([HID, 4, P], f16, tag="comb",
                                   name=f"cmb{s}_{gi}")
                nc.scalar.activation(combT[:, :gn, :], pcbs.pop(gi)[:, :gn, :],
                                     AF.Prelu, alpha=0.2)
                return combT

            combTs = {}

            def stage_att(gi):                # PE <- combT (1 group ago)
                gn, cs0 = g_n(gi), 4 * gi
                combT = combTs.pop(gi)
                for k in range(gn):
                    ci = cs0 + k
                    nc.tensor.matmul(out=ex_ps[:, ci, :], lhsT=combT[:, k, :],
                                     rhs=att_sb[:], start=True, stop=True,
                                     skip_group_check=True)

            def stage_exp(gi):                # Act <- ex_ps
                gn, cs0 = g_n(gi), 4 * gi
                nc.scalar.activation(rhs_t[:, cs0:cs0 + gn, 0:4],
                                     ex_ps[:, cs0:cs0 + gn, :], AF.Exp)

            def stage_wmul(gi):               # DVE/Pool <- exp, xs
                gn, cs0 = g_n(gi), 4 * gi
                for k in range(gn):
                    ci = cs0 + k
                    eng = (nc.gpsimd if (ci % g.wmul_pool_mod
                                         == g.wmul_pool_mod - 1)
                           else nc.vector)
                    eng.tensor_mul(
                        rhs_t[:, ci, 4:].rearrange("p (d h) -> p d h", d=HD),
                        xs_t[:, ci, :].rearrange("p (d h) -> p d h", d=HD),
                        rhs_t[:, ci, 0:4].unsqueeze(1)
                        .broadcast_to([P, HD, H]))

            psTs = {}
            for gi in range(ngrp + 2):
                if gi < ngrp:
                    psTs[gi] = stage_transpose(gi)
                    stage_copy(gi, psTs.pop(gi))
                if 1 <= gi <= ngrp:
                    stage_pcb(gi - 1)
                    combTs[gi - 1] = stage_prelu(gi - 1)
                if 2 <= gi:
                    stage_att(gi - 2)
                    stage_exp(gi - 2)
                    stage_wmul(gi - 2)

            # -- scatter into window accumulators
            win_t = psWin.tile([P, g.slab_w, 4 + HID], f32, space="PSUM",
                               tag="win", name=f"win{s}")
            n_per_win = [0] * nwin
            for (wl, h, slot) in chunks:
                n_per_win[wl] += 1
            # window-major scatter order: accumulation groups sharing a
            # PSUM bank must not interleave on hardware
            seen = [0] * nwin
            order = sorted(range(C), key=lambda ci: chunks[ci][0])
            for ci in order:
                wl = chunks[ci][0]
                first = seen[wl] == 0
                last = seen[wl] == n_per_win[wl] - 1
                seen[wl] += 1
                nc.tensor.matmul(out=win_t[:, wl, :], lhsT=oh_t[:, ci, :],
                                 rhs=rhs_t[:, ci, :], start=first, stop=last,
                                 skip_group_check=True)
            wins[s] = (win_t, n_per_win)

        def emit_flush(s):
            # ---- flush windows (emitted one slab late so these dependency-
            # waiting ops never park in front of the next slab's work on the
            # in-order per-engine sequencers). Elementwise work is batched
            # across the slab's windows. -------------------------------------
            sl = sched["slabs"][s]
            ws = sl["windows"]
            nwin = len(ws)
            win_t, n_per_win = wins.pop(s)
            assert all(n > 0 for n in n_per_win)
            # f32: raw exp-sums can exceed the f16 max before the
            # normalization divide
            win16 = fpool.tile([P, nwin, 4 + HID], f32, tag="win16",
                               name=f"w16_{s}")
            nc.scalar.activation(win16[:], win_t[:, :nwin, :], AF.Prelu,
                                 alpha=1.0)
            sums = fpool.tile([P, nwin, 4], f32, tag="sums")
            nc.vector.tensor_scalar(sums[:], win16[:, :, 0:4], 1e-12, None,
                                    OP.max)
            rec = fpool.tile([P, nwin, 4], f32, tag="rec")
            nc.vector.reciprocal(rec[:], sums[:])
            aggn = fpool.tile([P, nwin, HID], f16, tag="aggn")
            nc.vector.tensor_mul(
                aggn[:].rearrange("p w (d h) -> p w d h", d=HD),
                win16[:, :, 4:].rearrange("p w (d h) -> p w d h", d=HD),
                rec[:].unsqueeze(2).broadcast_to([P, nwin, HD, H]))
            psTf = psT4.tile([P, 4, P], f16, space="PSUM", tag="psT")
            for wl in range(nwin):
                nc.tensor.transpose(out=psTf[:, wl, :], in_=aggn[:, wl, :],
                                    identity=ident[:])
            aggT = fpool.tile([HID, nwin, P], f16, tag="aggT")
            nc.vector.tensor_copy(aggT[:], psTf[:, :nwin, :])
            po = psSm.tile([P, g.slab_w, HID], f32, space="PSUM", tag="pt")
            for wl in range(nwin):
                nc.tensor.matmul(out=po[:, wl, :], lhsT=aggT[:, wl, :],
                                 rhs=Wosc[:], start=True, stop=False,
                                 skip_group_check=True)
                nc.tensor.matmul(out=po[:, wl, :], lhsT=ones_row[:],
                                 rhs=beta16[:], start=False, stop=True,
                                 skip_group_check=True)
            y = fpool.tile([P, nwin, HID], f32, tag="y", name=f"y{s}")
            nc.vector.tensor_add(y[:], po[:, :nwin, :],
                                 resid16[:, ws[0]:ws[0] + nwin, :])
            bn_sl = fpool.tile([P, nwin, 2], f32, tag="bnsl", name=f"bns{s}")
            for wl in range(nwin):
                bnst = fpool.tile([P, 6], f32, tag="bnst")
                nc.vector.bn_stats(bnst[:], y[:, wl, :])
                nc.vector.bn_aggr(bn_sl[:, wl, :], bnst[:])
            # rstd = exp(-0.5 * ln(var + eps)): keeps the Act engine within
            # one activation-function table ({Prelu, Exp, Ln}; Sqrt would
            # force a table reload every slab)
            # rstd = 1/sqrt(var+eps) fully on DVE (quake seed + 3 Newton
            # steps, ~2e-7): Sqrt/Ln on Act would force an activation-table
            # reload every slab (only {Prelu, Exp} stay loaded).
            vr = fpool.tile([P, nwin], f32, tag="vr", name=f"vr{s}")
            nc.vector.tensor_scalar(vr[:], bn_sl[:, :, 1], EPS_LN, None,
                                    OP.add)
            qi = fpool.tile([P, nwin], mybir.dt.int32, tag="qi",
                            name=f"qi{s}")
            nc.vector.tensor_scalar(qi[:], vr[:].bitcast(mybir.dt.int32),
                                    1, None, OP.logical_shift_right)
            nc.vector.tensor_scalar(qi[:], qi[:], -1, 0x5f3759df,
                                    OP.mult, OP.add)
            rstd = fpool.tile([P, nwin], f32, tag="rstd", name=f"rs{s}")
            nc.vector.tensor_copy(rstd[:].bitcast(mybir.dt.int32), qi[:])
            rt = fpool.tile([P, nwin], f32, tag="rt", name=f"rt{s}")
            for _ in range(3):
                nc.vector.tensor_mul(rt[:], rstd[:], rstd[:])
                nc.vector.tensor_mul(rt[:], rt[:], vr[:])
                nc.vector.tensor_scalar(rt[:], rt[:], -0.5, 1.5,
                                        OP.mult, OP.add)
                nc.vector.tensor_mul(rstd[:], rstd[:], rt[:])
            yn16 = fpool.tile([P, nwin, HID], f16, tag="yn", name=f"yn{s}")
            for wl in range(nwin):
                nc.vector.scalar_tensor_tensor(
                    yn16[:, wl, :], y[:, wl, :], bn_sl[:, wl, 0:1],
                    rstd[:, wl:wl + 1].broadcast_to([P, HID]),
                    OP.subtract, OP.mult)
                if not sched["skip_norm"]:
                    nc.vector.tensor_mul(yn16[:, wl, :], yn16[:, wl, :],
                                         w_rep[:])
                    nc.vector.tensor_add(yn16[:, wl, :], yn16[:, wl, :],
                                         b_rep[:])
            nfull = sum(1 for w in ws if npc - w * P >= P)
            if nfull:
                nc.sync.dma_start(
                    out[ws[0] * P:ws[0] * P + nfull * P, :]
                    .rearrange("(w p) h -> p w h", p=P),
                    yn16[:, :nfull, :])
            for wl, w in enumerate(ws):
                rows = npc - w * P
                if rows < P:
                    nc.sync.dma_start(out[w * P:w * P + rows, :],
                                      yn16[:rows, wl, :])

        nslab = len(sched["slabs"])
        for s in range(min(g.lookahead, nslab)):
            emit_front(s)
        for s in range(nslab):
            if s + g.lookahead < nslab:
                emit_front(s + g.lookahead)
            emit_back(s)
            # flush of the previous slab goes AFTER this slab's back: its
            # first op waits on slab s-1's scatter, and emitting it earlier
            # would park the in-order Act/DVE queues ahead of slab s's work
            if s >= 1:
                emit_flush(s - 1)
        emit_flush(nslab - 1)

    nc.compile()
    return nc


# ---------------------------------------------------------------------------
# Full-input entry point: shard, compile (cached), run SPMD on 8 cores,
# gather the output shards.
# ---------------------------------------------------------------------------
_CACHE = {}


def kernel(**inputs):
    N = int(np.asarray(inputs["node_embeddings"]).shape[0])
    n_cores = 8
    g = Geo(N=N, n_cores=n_cores)

    sched, in_maps = host_prep(g, **{k: np.asarray(v) for k, v in inputs.items()})

    key = (N, sched["total_chunks"], tuple(int(x) for x in sched["caps"].ravel()),
           sched["skip_norm"])
    if key not in _CACHE:
        _CACHE[key] = build_program(g, sched)
    nc = _CACHE[key]

    from concourse.bass_utils import run_bass_kernel_spmd
    res = run_bass_kernel_spmd(nc, in_maps, core_ids=list(range(n_cores)))
    out = np.concatenate([res.results[c]["out"] for c in range(n_cores)], axis=0)
    return out.astype(np.float32)


# revision 25
# speedup vs baseline: 1.0054x; 1.0054x over previous
"""GATv2 graph layer Bass kernel for TRN2 (SPMD across 8 NeuronCores, no
collectives).

Design (v3): edges sorted by destination node and sharded across cores by dst
range. Each core builds ONE fp16 gather table in DRAM (xsrc = node_emb @
W_src, split in two halves so int16 gather indices fit), with rows remapped so
table writes use 1KB descriptors. Destination-side features are NOT gathered
from DRAM: x_dst for the core's own nodes lives in SBUF ([P, nw, HID] f16),
and per edge chunk (128 edges, one 128-node dst window) the dst contribution
is gathered by the TensorEngine via a transposed one-hot:

  oh[e, d]  = (dstr[e] == d)                       (DVE tensor_scalar)
  ohT[d, e] = transpose(oh)                        (PE transpose, f16 PSUM)
  psum_combT[hid, e] = xdst_win^T @ ohT            (lhsT=xdst_win, run-batched)
                     + emb8^T @ ohetT              (one matmul per group)
                     + xs_chunk^T                  (lhsT=xs, rhs=identity)
  combT16 = Prelu(psum_combT, alpha=0.2)           (Act)
  ex_ps[e, h] = combT16^T @ att_blk                (PE)
  ex16 = Exp(ex_ps)                                (Act, per group)
  wgt = xs * bcast(ex16)                           (DVE, 4 chunks per op)
  win_ps[dst, 4+HID] += oh^T @ [ex16 | wgt]        (PE scatter)

The hidden dim is stored d-major ((d,h) instead of (h,d), a host-side column
permutation of W_src/W_dst/emb/att/W_out) so the per-head broadcast of ex in
the weight-mul has a packed innermost dim — that enables the DVE 2x perf mode
and 4-chunk batching. FiLM (tanh) and the W_out column scaling are folded
into constants on the host, so the only device activation functions are
{Prelu, Exp, Ln} — all within one activation table (no reload thrash; the
LayerNorm rstd is exp(-0.5*ln(var+eps))).

Window flush (batched across the slab's windows): agg = sum(ex*xs)/sum(ex),
@Wosc + beta_eff, residual, LayerNorm via bn_stats/bn_aggr, f16 DMA out.
No max-subtraction in the softmax: logits are bounded so exp stays finite in
f32 (matches reference to ~1e-4).
"""
import numpy as np
from contextlib import ExitStack
from dataclasses import dataclass

import concourse.bass as bass
import concourse.tile as tile
from concourse import bacc, mybir
from concourse.masks import make_identity

P = 128
HID = 128
H = 4
HD = 32
NET = 8
EPS_LN = 1e-5
MAXCALL = 4096  # max idxs per dma_gather call
DEAD = -5.0     # dst_rel for padding slots (matches no one-hot column)


@dataclass
class Geo:
    N: int
    n_cores: int
    slab_w: int = 3     # windows per slab
    lookahead: int = 2  # slabs of input prefetch
    ohT_act_mod: int = 8    # every k-th group's ohT copy runs on Act
    oh_pool_mod: int = 4    # every k-th chunk's oh gen runs on Pool
    wmul_pool_mod: int = 3  # every k-th chunk's weight-mul runs on Pool

    @property
    def npc(self):
        return self.N // self.n_cores

    @property
    def nw(self):
        return (self.npc + P - 1) // P

    @property
    def nslab(self):
        return (self.nw + self.slab_w - 1) // self.slab_w

    @property
    def n_pad(self):   # padded node count (512 blocks)
        return ((self.N + 511) // 512) * 512

    @property
    def split(self):   # lo/hi table split on a 512 block boundary
        return (self.n_pad // 1024) * 512


def wrap_idx(idx, cols):
    n = idx.shape[0]
    assert n % 16 == 0
    w = np.zeros((P, cols), dtype=np.int16)
    if n:
        t16 = idx.reshape(n // 16, 16).T
        for g in range(8):
            w[g * 16:(g + 1) * 16, :n // 16] = t16
    return w


def remap_row(r):
    """Table row remap so device-side table writes are 1KB/partition:
    original row i*512 + s*128 + p is stored at i*512 + p*4 + s."""
    i, rem = r // 512, r % 512
    return i * 512 + (rem % 128) * 4 + rem // 128


def host_prep(g: Geo, node_embeddings, edge_index, edge_type, task_embedding,
              W_src, b_src, W_dst, b_dst, edge_emb, att,
              W_out, b_out, norm_w, norm_b, W_film, b_film):
    """Returns (sched, in_maps). Pure index work + tiny constant folding;
    all O(N*HID) / O(E*HID) float math runs on device."""
    src = np.asarray(edge_index[0], dtype=np.int64)
    dst = np.asarray(edge_index[1], dtype=np.int64)
    et = np.asarray(edge_type, dtype=np.int64)
    npc, split = g.npc, g.split

    order = np.argsort(dst, kind="stable")
    src, dst, et = src[order], dst[order], et[order]
    core_of = dst // npc

    # Per-core window balancing: permute each core's own nodes across its
    # windows so per-(window, half) edge counts are near-equal across all
    # cores. caps[w,h] is a max over cores, so imbalance pads every core's
    # chunk schedule; balancing cuts total chunks ~15% (pure host index
    # work; the output rows are un-permuted on the host afterwards).
    w_caps = np.array([min(P, npc - w * P) for w in range(g.nw)])
    poss, invs = [], []
    buckets = {}
    for c in range(g.n_cores):
        m = core_of == c
        cs, cd, ce = src[m], dst[m] - c * npc, et[m]
        lo_m = cs < split
        lo_deg = np.bincount(cd[lo_m], minlength=npc).astype(np.float64)
        hi_deg = np.bincount(cd[~lo_m], minlength=npc).astype(np.float64)
        order = np.argsort(-(lo_deg + hi_deg), kind="stable")
        Llo = np.zeros(g.nw)
        Lhi = np.zeros(g.nw)
        nfill = np.zeros(g.nw, dtype=np.int64)
        pos = np.empty(npc, dtype=np.int64)
        # objective = chunk count: stay within 5*P edges per half where
        # possible; unavoidable overflow chunks concentrate in the highest
        # windows (same ones on every core -> shared caps stay tight)
        TGT = 5 * P
        wrank = (g.nw - np.arange(g.nw)) * 1e6
        for node in order:
            costl = Llo + lo_deg[node]
            costh = Lhi + hi_deg[node]
            extra = (np.ceil(np.maximum(costl, TGT) / P) - 5
                     + np.ceil(np.maximum(costh, TGT) / P) - 5)
            key = extra * 1e9 + np.where(extra > 0, wrank, 0.0) \
                + np.maximum(costl, costh)
            key[nfill >= w_caps] = np.inf
            w = int(np.argmin(key))
            pos[node] = w * P + nfill[w]
            nfill[w] += 1
            Llo[w] += lo_deg[node]
            Lhi[w] += hi_deg[node]
        # swap-repair: the greedy's endgame is slot-capacity-bound, leaving
        # a few windows a couple of edges over TGT; pairwise node swaps with
        # slack windows remove (or concentrate in the last windows) the
        # overflow so the shared caps stay at 5 chunks per half
        members = [[] for _ in range(g.nw)]
        for node in range(npc):
            members[pos[node] // P].append(node)
        for _ in range(400):
            over = np.maximum(Llo - TGT, 0) + np.maximum(Lhi - TGT, 0)
            over[g.nw - 1] = 0.0   # tail window is the designated spill
            wb = int(np.argmax(over))
            if over[wb] <= 0:
                break
            def do_swap(a, b, u):
                members[wb].remove(a)
                members[u].remove(b)
                members[wb].append(b)
                members[u].append(a)
                pos[a], pos[b] = pos[b], pos[a]
                Llo[wb] += lo_deg[b] - lo_deg[a]
                Lhi[wb] += hi_deg[b] - hi_deg[a]
                Llo[u] += lo_deg[a] - lo_deg[b]
                Lhi[u] += hi_deg[a] - hi_deg[b]

            done = False
            cand_a = sorted(members[wb],
                            key=lambda n: -(lo_deg[n] + hi_deg[n]))[:12]
            for u in np.argsort(np.maximum(Llo, Lhi)):
                if u == wb:
                    continue
                for a in cand_a:
                    la, ha = lo_deg[a], hi_deg[a]
                    bs = members[u]
                    lb = lo_deg[bs]
                    hb = hi_deg[bs]
                    ok = ((Llo[wb] - la + lb <= TGT)
                          & (Lhi[wb] - ha + hb <= TGT)
                          & (Llo[u] + la - lb <= TGT)
                          & (Lhi[u] + ha - hb <= TGT))
                    if ok.any():
                        do_swap(a, bs[int(np.argmax(ok))], u)
                        done = True
                        break
                if done:
                    break
            if not done:
                # totals force overflow somewhere: concentrate it in the
                # tail window (same window on every core)
                u = g.nw - 1
                a = cand_a[0]
                bs = members[u]
                j = int(np.argmin(lo_deg[bs] + hi_deg[bs]))
                b = bs[j]
                if lo_deg[a] + hi_deg[a] > lo_deg[b] + hi_deg[b]:
                    do_swap(a, b, u)
                else:
                    break
        # only the last window is short, so slots coincide with compact rows
        assert pos.max() < npc
        inv = np.empty(npc, dtype=np.int64)
        inv[pos] = np.arange(npc)    # new row -> old local row
        poss.append(pos)
        invs.append(inv)

        cdp = pos[cd]                # edge dst -> new row (window-aligned)
        for w in range(g.nw):
            wm = (cdp // P) == w
            ws_, wd, we = cs[wm], cdp[wm] - w * P, ce[wm]
            lo = ws_ < split
            buckets[(c, w, 0)] = (remap_row(ws_[lo]), wd[lo], we[lo])
            buckets[(c, w, 1)] = (remap_row(ws_[~lo] - split), wd[~lo], we[~lo])

    caps = np.zeros((g.nw, 2), dtype=np.int64)
    for w in range(g.nw):
        for h in range(2):
            mx = max(len(buckets[(c, w, h)][0]) for c in range(g.n_cores))
            caps[w, h] = (mx + P - 1) // P

    # ---- schedule ---------------------------------------------------------
    sched_slabs = []
    total_chunks = 0
    for s in range(g.nslab):
        ws = list(range(s * g.slab_w, min((s + 1) * g.slab_w, g.nw)))
        chunks = []            # (win_local, half, slot)
        calls = {0: [], 1: []}
        slot = 0
        for h in (0, 1):
            run = 0
            run_start = slot
            for w in ws:
                for _ in range(caps[w, h]):
                    chunks.append((w - ws[0], h, slot))
                    slot += 1
                    run += P
                    if run == MAXCALL:
                        calls[h].append((run_start, run))
                        run, run_start = 0, slot
            if run:
                calls[h].append((run_start, run))
        sched_slabs.append(dict(windows=ws, chunks=chunks, calls=calls,
                                chunk0=total_chunks))
        total_chunks += len(chunks)

    lo_cols = max(16, sum(n for sl in sched_slabs
                          for (_, n) in sl["calls"][0]) // 16)
    hi_cols = max(16, sum(n for sl in sched_slabs
                          for (_, n) in sl["calls"][1]) // 16)

    # ---- shared constants -------------------------------------------------
    # d-major permutation of the hidden dim: new col j holds old col
    # (j%H)*HD + j//H, i.e. feature (h, d) lives at j = d*H + h.
    perm = np.array([(j % H) * HD + j // H for j in range(HID)], dtype=np.int64)

    nodeT = np.zeros((HID, g.n_pad), dtype=np.float16)
    nodeT[:, :g.N] = np.asarray(node_embeddings, np.float32).T.astype(np.float16)
    emb_eff = (np.asarray(edge_emb, np.float64)
               + np.asarray(b_src, np.float64)[None, :]
               + np.asarray(b_dst, np.float64)[None, :])[:, perm].astype(np.float16)
    att_blk = np.zeros((HID, H), dtype=np.float16)
    att64 = np.asarray(att, np.float64)
    for h in range(H):
        for d in range(HD):
            att_blk[d * H + h, h] = att64[h, d]

    # FiLM fold (O(HID^2) host math): gamma/beta modulation of the output
    # projection becomes a column scale on W_out plus a bias.
    film = (np.asarray(task_embedding, np.float64)
            @ np.asarray(W_film, np.float64)
            + np.asarray(b_film, np.float64))
    gm = 1.0 + 0.5 * np.tanh(film[:HID])
    beta_eff = np.asarray(b_out, np.float64) * gm + film[HID:]
    Wosc = (np.asarray(W_out, np.float64)[perm, :] * gm[None, :])

    consts = dict(
        nodeT=nodeT,
        W_src=np.asarray(W_src, np.float32)[:, perm].astype(np.float16),
        W_dst=np.asarray(W_dst, np.float32)[:, perm].astype(np.float16),
        Wosc=Wosc.astype(np.float16),
        beta=beta_eff.astype(np.float16).reshape(1, HID),
        emb8=emb_eff,                       # [8, HID] f16 (permuted)
        att_blk=att_blk,
    )
    skip_norm = bool(np.all(np.asarray(norm_w) == 1.0)
                     and np.all(np.asarray(norm_b) == 0.0))
    if not skip_norm:
        consts["normw"] = np.asarray(norm_w, np.float32).reshape(1, HID)
        consts["normb"] = np.asarray(norm_b, np.float32).reshape(1, HID)

    node_f16 = np.asarray(node_embeddings, np.float32).astype(np.float16)

    # ---- per-core arrays --------------------------------------------------
    in_maps = []
    for c in range(g.n_cores):
        lo_l, hi_l = [], []
        dstr = np.full((P, total_chunks), DEAD, dtype=np.float32)
        oet = np.zeros((NET, total_chunks * P), dtype=np.float16)
        ci = 0
        for sl in sched_slabs:
            ws0 = sl["windows"][0]
            per_half = {0: [], 1: []}
            nth = {}
            for (wl, h, slot) in sl["chunks"]:
                w = ws0 + wl
                es, ed, ee = buckets[(c, w, h)]
                k = nth.get((wl, h), 0)
                nth[(wl, h)] = k + 1
                sl_src = np.zeros(P, dtype=np.int64)
                n = min(P, max(0, len(es) - k * P))
                if n > 0:
                    sl_src[:n] = es[k * P:k * P + n]
                    dstr[:n, ci] = ed[k * P:k * P + n]
                    oet[ee[k * P:k * P + n], ci * P + np.arange(n)] = 1.0
                per_half[h].append(sl_src)
                ci += 1
            lo_l.extend(per_half[0])
            hi_l.extend(per_half[1])
        lo_i = (np.concatenate(lo_l) if lo_l else np.zeros(0, np.int64))
        hi_i = (np.concatenate(hi_l) if hi_l else np.zeros(0, np.int64))
        assert lo_i.max(initial=0) < g.split <= 32767
        assert hi_i.max(initial=0) < g.n_pad - g.split <= 32768

        own = node_f16[c * npc:(c + 1) * npc][invs[c]]  # [npc, HID] permuted
        ownT = np.zeros((HID, g.nw * P), dtype=np.float16)
        ownT[:, :npc] = own.T

        m = dict(consts)
        m["node_own16"] = np.ascontiguousarray(own)
        m["node_ownT"] = ownT
        m["lo_idx"] = wrap_idx(lo_i.astype(np.int16), lo_cols)
        m["hi_idx"] = wrap_idx(hi_i.astype(np.int16), hi_cols)
        m["dstr"] = dstr
        m["ohetT"] = oet
        in_maps.append(m)

    sched = dict(slabs=sched_slabs, caps=caps, total_chunks=total_chunks,
                 lo_cols=lo_cols, hi_cols=hi_cols, skip_norm=skip_norm,
                 poss=poss)
    return sched, in_maps


def build_program(g: Geo, sched, debug=False):
    nc = bacc.Bacc("TRN2", target_bir_lowering=False, debug=False,
                   num_devices=g.n_cores, num_swdge_queues=4)
    f16, f32 = mybir.dt.float16, mybir.dt.float32
    AF = mybir.ActivationFunctionType
    OP = mybir.AluOpType
    npc, nw = g.npc, g.nw
    total_chunks = sched["total_chunks"]
    lo_cols, hi_cols = sched["lo_cols"], sched["hi_cols"]

    def din(name, shape, dt):
        return nc.dram_tensor(name, shape, dt, kind="ExternalInput").ap()

    nodeT = din("nodeT", [HID, g.n_pad], f16)
    node_own16 = din("node_own16", [npc, HID], f16)
    node_ownT = din("node_ownT", [HID, nw * P], f16)
    W_src = din("W_src", [HID, HID], f16)
    W_dst = din("W_dst", [HID, HID], f16)
    Wosc_dr = din("Wosc", [HID, HID], f16)
    beta_dr = din("beta", [1, HID], f16)
    emb8 = din("emb8", [NET, HID], f16)
    att_blk = din("att_blk", [HID, H], f16)
    lo_idx = din("lo_idx", [P, lo_cols], mybir.dt.int16)
    hi_idx = din("hi_idx", [P, hi_cols], mybir.dt.int16)
    dstr = din("dstr", [P, total_chunks], f32)
    ohetT = din("ohetT", [NET, total_chunks * P], f16)
    out = nc.dram_tensor("out", [npc, HID], f16, kind="ExternalOutput").ap()

    xsrc_tab = nc.dram_tensor("xsrc_tab", [g.n_pad, HID], f16,
                              kind="Internal").ap()

    with tile.TileContext(nc, trace_sim=False) as tc, ExitStack() as ctx:
        cpool = ctx.enter_context(tc.tile_pool(name="consts", bufs=1))
        bpool = ctx.enter_context(tc.tile_pool(name="build", bufs=3))
        # PSUM: 8 banks of 2KB/partition, one per tile buf. Exactly 8 bufs.
        psBig = ctx.enter_context(tc.tile_pool(name="psBig", bufs=2,
                                               space="PSUM"))
        psT4 = ctx.enter_context(tc.tile_pool(name="psT4", bufs=2,
                                              space="PSUM"))
        psWin = ctx.enter_context(tc.tile_pool(name="psWin", bufs=2,
                                               space="PSUM"))
        psEx = ctx.enter_context(tc.tile_pool(name="psEx", bufs=1,
                                              space="PSUM"))
        psSm = ctx.enter_context(tc.tile_pool(name="psSm", bufs=1,
                                              space="PSUM"))
        spool = ctx.enter_context(tc.tile_pool(name="slab",
                                               bufs=g.lookahead + 1))
        gpool = ctx.enter_context(tc.tile_pool(name="grp", bufs=3))
        fpool = ctx.enter_context(tc.tile_pool(name="flush", bufs=2))

        # ---- idx + dstr staging (first: gathers only wait on the table) ---
        lo_sb = cpool.tile([P, lo_cols], mybir.dt.int16, tag="loidx")
        nc.sync.dma_start(lo_sb[:], lo_idx[:])
        hi_sb = cpool.tile([P, hi_cols], mybir.dt.int16, tag="hiidx")
        nc.sync.dma_start(hi_sb[:], hi_idx[:])
        dstr_sb = cpool.tile([P, total_chunks], f32, tag="dstr")
        nc.sync.dma_start(dstr_sb[:], dstr[:])

        # ---- constants ----------------------------------------------------
        ident = cpool.tile([P, P], f16)
        make_identity(nc, ident[:])
        iota16 = cpool.tile([P, P], mybir.dt.int16)
        nc.gpsimd.iota(iota16[:], pattern=[[1, P]], base=0, channel_multiplier=0)
        iota = cpool.tile([P, P], f16)
        nc.vector.tensor_copy(iota[:], iota16[:])
        ones_row = cpool.tile([1, P], f16)
        nc.vector.memset(ones_row[:], 1.0)
        eps_col = cpool.tile([P, 1], f32)
        nc.vector.memset(eps_col[:], EPS_LN)

        Ws = cpool.tile([HID, HID], f16)
        nc.sync.dma_start(Ws[:], W_src[:])
        Wd = cpool.tile([HID, HID], f16)
        nc.sync.dma_start(Wd[:], W_dst[:])
        Wosc = cpool.tile([HID, HID], f16)
        nc.sync.dma_start(Wosc[:], Wosc_dr[:])
        beta16 = cpool.tile([1, HID], f16)
        nc.sync.dma_start(beta16[:], beta_dr[:])
        emb_sb = cpool.tile([NET, HID], f16)
        nc.sync.dma_start(emb_sb[:], emb8[:])
        att_sb = cpool.tile([HID, H], f16)
        nc.sync.dma_start(att_sb[:], att_blk[:])

        # residual (own node embeddings) as [P, nw, HID] f16
        resid16 = cpool.tile([P, nw, HID], f16, tag="resid")
        tail = npc - (npc // P) * P
        full_w = npc // P
        if tail:
            nc.vector.memset(resid16[:, full_w, :], 0.0)
        if full_w:
            nc.sync.dma_start(
                resid16[:, :full_w, :],
                node_own16[:full_w * P, :].rearrange("(w p) h -> p w h", p=P))
        if tail:
            nc.sync.dma_start(resid16[:tail, full_w, :],
                              node_own16[full_w * P:, :])

        if not sched["skip_norm"]:
            nw_dr = din("normw", [1, HID], f32)
            nb_dr = din("normb", [1, HID], f32)
            nw_sb = cpool.tile([1, HID], f32)
            nc.sync.dma_start(nw_sb[:], nw_dr[:])
            nb_sb = cpool.tile([1, HID], f32)
            nc.sync.dma_start(nb_sb[:], nb_dr[:])
            ones32 = cpool.tile([1, P], f32)
            nc.vector.memset(ones32[:], 1.0)
            ps_w = psBig.tile([P, HID], f32, space="PSUM", tag="pcb")
            nc.tensor.matmul(out=ps_w[:], lhsT=ones32[:], rhs=nw_sb[:],
                             start=True, stop=True)
            w_rep = cpool.tile([P, HID], f32)
            nc.vector.tensor_copy(w_rep[:], ps_w[:])
            ps_b = psBig.tile([P, HID], f32, space="PSUM", tag="pcb")
            nc.tensor.matmul(out=ps_b[:], lhsT=ones32[:], rhs=nb_sb[:],
                             start=True, stop=True)
            b_rep = cpool.tile([P, HID], f32)
            nc.vector.tensor_copy(b_rep[:], ps_b[:])

        # ---- xdst table (SBUF-resident) -----------------------------------
        xdst_sb = cpool.tile([P, nw, HID], f16, tag="xdst")
        for gl in range(0, nw, 16):
            gln = min(16, nw - gl)
            not_t = bpool.tile([HID, 16 * P], f16, tag="not")
            nc.sync.dma_start(not_t[:, :gln * P],
                              node_ownT[:, gl * P:(gl + gln) * P])
            for gw in range(gl, gl + gln, 4):
                gn = min(4, gl + gln - gw)
                psX = psBig.tile([P, 4, HID], f32, space="PSUM", tag="pcb")
                for k in range(gn):
                    nc.tensor.matmul(out=psX[:, k, :],
                                     lhsT=not_t[:, (gw - gl + k) * P:
                                                (gw - gl + k + 1) * P],
                                     rhs=Wd[:],
                                     start=True, stop=True,
                                     skip_group_check=True)
                nc.scalar.activation(xdst_sb[:, gw:gw + gn, :], psX[:, :gn, :],
                                     AF.Prelu, alpha=1.0)

        # ---- xsrc gather table --------------------------------------------
        # 2048-node iterations: one big nodeT load (SP) + one big table write
        # (Act) each -- HWDGE charges ~0.6us per DMA instruction, so few big
        # DMAs beat many small ones. PSUM->SBUF copies alternate DVE/Act.
        starts = list(range(0, g.n_pad, 2048))  # last block may be partial
        nts = []
        for i, st in enumerate(starts):
            wdt = min(2048, g.n_pad - st)
            nt = bpool.tile([HID, 2048], f16, tag="nt", name=f"nt{i}",
                            bufs=3)
            nc.sync.dma_start(nt[:, :wdt], nodeT[:, st:st + wdt])
            nts.append(nt)
        for i, st in enumerate(starts):
            wdt = min(2048, g.n_pad - st)
            nq = wdt // 512
            nt = nts[i]
            xt = bpool.tile([P, 4, 4, HID], f16, tag="xt", bufs=3)
            for q in range(nq):
                ps = psBig.tile([P, 4, HID], f32, space="PSUM", tag="pcb")
                for j in range(4):
                    nc.tensor.matmul(out=ps[:, j, :],
                                     lhsT=nt[:, q * 512 + j * P:
                                             q * 512 + (j + 1) * P],
                                     rhs=Ws[:],
                                     start=True, stop=True,
                                     skip_group_check=True)
                if q % 2 == 0:
                    nc.vector.tensor_copy(xt[:, q, :, :], ps[:])
                else:
                    nc.scalar.activation(xt[:, q, :, :], ps[:],
                                         AF.Prelu, alpha=1.0)
            # dram row st + q*512 + p*4 + s  <-  xt[p, q, s, :]
            nc.scalar.dma_start(
                xsrc_tab[st:st + wdt, :]
                .rearrange("(q p s) h -> p q s h", p=P, s=4),
                xt[:, :nq, :, :])

        off16 = {0: 0, 1: 0}
        qn = {0: 0, 1: 0}

        def nextq(h):
            # dedicated queues per table half: a hi gather parked on the full
            # table must not block later lo gathers (lo half is ready first)
            qn[h] ^= 1
            return 2 * h + qn[h]

        # ---- edge slabs ----------------------------------------------------
        # Software-pipelined emission: each slab's input loads and one-hot
        # generation ("front") are emitted `lookahead` slabs ahead of its
        # compute + flush ("back"), so the in-order per-engine sequencers
        # never park a next-slab load behind a previous slab's dependency
        # waits.
        fronts = {}
        wins = {}

        def emit_front(s):
            sl = sched["slabs"][s]
            chunks = sl["chunks"]
            C = len(chunks)
            c0 = sl["chunk0"]

            xs_t = spool.tile([P, C, HID], f16, tag="xs", name=f"xs{s}")
            for h in (0, 1):
                base = 0 if h == 0 else g.split
                lim = g.split if h == 0 else g.n_pad
                idx_sb = lo_sb if h == 0 else hi_sb
                for (slot_off, n) in sl["calls"][h]:
                    if n == 0:
                        continue
                    nc.gpsimd.dma_gather(
                        out_ap=xs_t[:, slot_off:slot_off + n // P, :],
                        in_ap=xsrc_tab[base:lim, :],
                        idxs_ap=idx_sb[:, off16[h]:off16[h] + n // 16],
                        num_idxs=n, num_idxs_reg=n, elem_size=HID,
                        single_packet=(n <= 1024), queue_num=nextq(h),
                    )
                    off16[h] += n // 16

            oet_t = spool.tile([NET, C * P], f16, tag="oet", name=f"oet{s}")
            nc.sync.dma_start(oet_t[:], ohetT[:, c0 * P:(c0 + C) * P])

            oh_t = spool.tile([P, C, P], f16, tag="oh", name=f"oh{s}")
            for ci in range(C):
                eng = (nc.gpsimd if ci % g.oh_pool_mod == g.oh_pool_mod - 1
                       else nc.vector)
                eng.tensor_scalar(
                    oh_t[:, ci, :], iota[:], dstr_sb[:, c0 + ci:c0 + ci + 1],
                    None, OP.is_equal)
            fronts[s] = (xs_t, oet_t, oh_t)

        def emit_back(s):
            sl = sched["slabs"][s]
            ws = sl["windows"]
            nwin = len(ws)
            chunks = sl["chunks"]
            C = len(chunks)
            c0 = sl["chunk0"]
            xs_t, oet_t, oh_t = fronts.pop(s)

            ohT_t = spool.tile([P, C, P], f16, tag="ohT", name=f"ohT{s}",
                               bufs=2)
            rhs_t = spool.tile([P, C, 4 + HID], f16, tag="rhs",
                               name=f"rhs{s}", bufs=2)
            ex_ps = psEx.tile([P, C, H], f32, space="PSUM", tag="ex")

            # Group-level software pipeline (stagger 1): the PE sequencer is
            # in-order and the cost model's DVFS ramp punishes every PE stall,
            # so each PE stage must only consume results produced >= 1 group
            # earlier on the other engines.
            ngrp = (C + 3) // 4

            def g_n(gi):
                return min(4, C - 4 * gi)

            pcbs = {}

            def stage_transpose(gi):          # PE <- oh (front, ready)
                gn, cs0 = g_n(gi), 4 * gi
                psT = psT4.tile([P, 4, P], f16, space="PSUM", tag="psT",
                                name=f"psT{s}_{gi}")
                for k in range(gn):
                    nc.tensor.transpose(out=psT[:, k, :],
                                        in_=oh_t[:, cs0 + k, :],
                                        identity=ident[:])
                return psT

            def stage_copy(gi, psT):          # DVE/Act <- psT
                gn, cs0 = g_n(gi), 4 * gi
                if gi % g.ohT_act_mod == g.ohT_act_mod - 1:
                    nc.scalar.activation(ohT_t[:, cs0:cs0 + gn, :],
                                         psT[:, :gn, :], AF.Prelu, alpha=1.0)
                else:
                    nc.vector.tensor_copy(ohT_t[:, cs0:cs0 + gn, :],
                                          psT[:, :gn, :])

            def stage_pcb(gi):                # PE <- ohT copy (1 group ago)
                gn, cs0 = g_n(gi), 4 * gi
                pcb = psBig.tile([HID, 4, P], f32, space="PSUM", tag="pcb",
                                 name=f"pcb{s}_{gi}")
                for k in range(gn):
                    ci = cs0 + k
                    wl = chunks[ci][0]
                    nc.tensor.matmul(out=pcb[:, k, :],
                                     lhsT=xdst_sb[:, ws[wl], :],
                                     rhs=ohT_t[:, ci, :],
                                     start=True, stop=False,
                                     skip_group_check=True)
                    nc.tensor.matmul(out=pcb[:, k, :], lhsT=emb_sb[:],
                                     rhs=oet_t[:, ci * P:(ci + 1) * P],
                                     start=False, stop=False,
                                     skip_group_check=True)
                    nc.tensor.matmul(out=pcb[:, k, :], lhsT=xs_t[:, ci, :],
                                     rhs=ident[:], start=False, stop=True,
                                     skip_group_check=True)
                pcbs[gi] = pcb

            def stage_prelu(gi):              # Act <- pcb
                gn, cs0 = g_n(gi), 4 * gi
                combT = gpool.tile([HID, 4, P], f16, tag="comb",
                                   name=f"cmb{s}_{gi}")
                nc.scalar.activation(combT[:, :gn, :], pcbs.pop(gi)[:, :gn, :],
                                     AF.Prelu, alpha=0.2)
                return combT

            combTs = {}

            def stage_att(gi):                # PE <- combT (1 group ago)
                gn, cs0 = g_n(gi), 4 * gi
                combT = combTs.pop(gi)
                for k in range(gn):
                    ci = cs0 + k
                    nc.tensor.matmul(out=ex_ps[:, ci, :], lhsT=combT[:, k, :],
                                     rhs=att_sb[:], start=True, stop=True,
                                     skip_group_check=True)

            def stage_exp(gi):                # Act <- ex_ps
                gn, cs0 = g_n(gi), 4 * gi
                nc.scalar.activation(rhs_t[:, cs0:cs0 + gn, 0:4],
                                     ex_ps[:, cs0:cs0 + gn, :], AF.Exp)

            def stage_wmul(gi):               # DVE/Pool <- exp, xs
                gn, cs0 = g_n(gi), 4 * gi
                for k in range(gn):
                    ci = cs0 + k
                    eng = (nc.gpsimd if (ci % g.wmul_pool_mod
                                         == g.wmul_pool_mod - 1)
                           else nc.vector)
                    eng.tensor_mul(
                        rhs_t[:, ci, 4:].rearrange("p (d h) -> p d h", d=HD),
                        xs_t[:, ci, :].rearrange("p (d h) -> p d h", d=HD),
                        rhs_t[:, ci, 0:4].unsqueeze(1)
                        .broadcast_to([P, HD, H]))

            psTs = {}
            for gi in range(ngrp + 2):
                if gi < ngrp:
                    psTs[gi] = stage_transpose(gi)
                    stage_copy(gi, psTs.pop(gi))
                if 1 <= gi <= ngrp:
                    stage_pcb(gi - 1)
                    combTs[gi - 1] = stage_prelu(gi - 1)
                if 2 <= gi:
                    stage_att(gi - 2)
                    stage_exp(gi - 2)
                    stage_wmul(gi - 2)

            # -- scatter into window accumulators
            win_t = psWin.tile([P, g.slab_w, 4 + HID], f32, space="PSUM",
                               tag="win", name=f"win{s}")
            n_per_win = [0] * nwin
            for (wl, h, slot) in chunks:
                n_per_win[wl] += 1
            # window-major scatter order: accumulation groups sharing a
            # PSUM bank must not interleave on hardware
            seen = [0] * nwin
            order = sorted(range(C), key=lambda ci: chunks[ci][0])
            for ci in order:
                wl = chunks[ci][0]
                first = seen[wl] == 0
                last = seen[wl] == n_per_win[wl] - 1
                seen[wl] += 1
                nc.tensor.matmul(out=win_t[:, wl, :], lhsT=oh_t[:, ci, :],
                                 rhs=rhs_t[:, ci, :], start=first, stop=last,
                                 skip_group_check=True)
            wins[s] = (win_t, n_per_win)

        def emit_flush(s):
            # ---- flush windows (emitted one slab late so these dependency-
            # waiting ops never park in front of the next slab's work on the
            # in-order per-engine sequencers). Elementwise work is batched
            # across the slab's windows. -------------------------------------
            sl = sched["slabs"][s]
            ws = sl["windows"]
            nwin = len(ws)
            win_t, n_per_win = wins.pop(s)
            assert all(n > 0 for n in n_per_win)
            # f32: raw exp-sums can exceed the f16 max before the
            # normalization divide
            win16 = fpool.tile([P, nwin, 4 + HID], f32, tag="win16",
                               name=f"w16_{s}")
            nc.scalar.activation(win16[:], win_t[:, :nwin, :], AF.Prelu,
                                 alpha=1.0)
            sums = fpool.tile([P, nwin, 4], f32, tag="sums")
            nc.vector.tensor_scalar(sums[:], win16[:, :, 0:4], 1e-12, None,
                                    OP.max)
            rec = fpool.tile([P, nwin, 4], f32, tag="rec")
            nc.vector.reciprocal(rec[:], sums[:])
            aggn = fpool.tile([P, nwin, HID], f16, tag="aggn")
            nc.vector.tensor_mul(
                aggn[:].rearrange("p w (d h) -> p w d h", d=HD),
                win16[:, :, 4:].rearrange("p w (d h) -> p w d h", d=HD),
                rec[:].unsqueeze(2).broadcast_to([P, nwin, HD, H]))
            psTf = psT4.tile([P, 4, P], f16, space="PSUM", tag="psT")
            for wl in range(nwin):
                nc.tensor.transpose(out=psTf[:, wl, :], in_=aggn[:, wl, :],
                                    identity=ident[:])
            aggT = fpool.tile([HID, nwin, P], f16, tag="aggT")
            nc.vector.tensor_copy(aggT[:], psTf[:, :nwin, :])
            po = psSm.tile([P, g.slab_w, HID], f32, space="PSUM", tag="pt")
            for wl in range(nwin):
                nc.tensor.matmul(out=po[:, wl, :], lhsT=aggT[:, wl, :],
                                 rhs=Wosc[:], start=True, stop=False,
                                 skip_group_check=True)
                nc.tensor.matmul(out=po[:, wl, :], lhsT=ones_row[:],
                                 rhs=beta16[:], start=False, stop=True,
                                 skip_group_check=True)
            y = fpool.tile([P, nwin, HID], f32, tag="y", name=f"y{s}")
            nc.vector.tensor_add(y[:], po[:, :nwin, :],
                                 resid16[:, ws[0]:ws[0] + nwin, :])
            bn_sl = fpool.tile([P, nwin, 2], f32, tag="bnsl", name=f"bns{s}")
            for wl in range(nwin):
                bnst = fpool.tile([P, 6], f32, tag="bnst")
                nc.vector.bn_stats(bnst[:], y[:, wl, :])
                nc.vector.bn_aggr(bn_sl[:, wl, :], bnst[:])
            # rstd = exp(-0.5 * ln(var + eps)): keeps the Act engine within
            # one activation-function table ({Prelu, Exp, Ln}; Sqrt would
            # force a table reload every slab)
            # rstd = 1/sqrt(var+eps) fully on DVE (quake seed + 3 Newton
            # steps, ~2e-7): Sqrt/Ln on Act would force an activation-table
            # reload every slab (only {Prelu, Exp} stay loaded).
            vr = fpool.tile([P, nwin], f32, tag="vr", name=f"vr{s}")
            nc.vector.tensor_scalar(vr[:], bn_sl[:, :, 1], EPS_LN, None,
                                    OP.add)
            qi = fpool.tile([P, nwin], mybir.dt.int32, tag="qi",
                            name=f"qi{s}")
            nc.vector.tensor_scalar(qi[:], vr[:].bitcast(mybir.dt.int32),
                                    1, None, OP.logical_shift_right)
            nc.vector.tensor_scalar(qi[:], qi[:], -1, 0x5f3759df,
                                    OP.mult, OP.add)
            rstd = fpool.tile([P, nwin], f32, tag="rstd", name=f"rs{s}")
            nc.vector.tensor_copy(rstd[:].bitcast(mybir.dt.int32), qi[:])
            rt = fpool.tile([P, nwin], f32, tag="rt", name=f"rt{s}")
            for _ in range(3):
                nc.vector.tensor_mul(rt[:], rstd[:], rstd[:])
                nc.vector.tensor_mul(rt[:], rt[:], vr[:])
                nc.vector.tensor_scalar(rt[:], rt[:], -0.5, 1.5,
                                        OP.mult, OP.add)
                nc.vector.tensor_mul(rstd[:], rstd[:], rt[:])
            yn16 = fpool.tile([P, nwin, HID], f16, tag="yn", name=f"yn{s}")
            for wl in range(nwin):
                nc.vector.scalar_tensor_tensor(
                    yn16[:, wl, :], y[:, wl, :], bn_sl[:, wl, 0:1],
                    rstd[:, wl:wl + 1].broadcast_to([P, HID]),
                    OP.subtract, OP.mult)
                if not sched["skip_norm"]:
                    nc.vector.tensor_mul(yn16[:, wl, :], yn16[:, wl, :],
                                         w_rep[:])
                    nc.vector.tensor_add(yn16[:, wl, :], yn16[:, wl, :],
                                         b_rep[:])
            nfull = sum(1 for w in ws if npc - w * P >= P)
            if nfull:
                nc.sync.dma_start(
                    out[ws[0] * P:ws[0] * P + nfull * P, :]
                    .rearrange("(w p) h -> p w h", p=P),
                    yn16[:, :nfull, :])
            for wl, w in enumerate(ws):
                rows = npc - w * P
                if rows < P:
                    nc.sync.dma_start(out[w * P:w * P + rows, :],
                                      yn16[:rows, wl, :])

        nslab = len(sched["slabs"])
        for s in range(min(g.lookahead, nslab)):
            emit_front(s)
        for s in range(nslab):
            if s + g.lookahead < nslab:
                emit_front(s + g.lookahead)
            emit_back(s)
            # flush of the previous slab goes AFTER this slab's back: its
            # first op waits on slab s-1's scatter, and emitting it earlier
            # would park the in-order Act/DVE queues ahead of slab s's work
            if s >= 1:
                emit_flush(s - 1)
        emit_flush(nslab - 1)

    nc.compile()
    return nc


# ---------------------------------------------------------------------------
# Full-input entry point: shard, compile (cached), run SPMD on 8 cores,
# gather the output shards.
# ---------------------------------------------------------------------------
_CACHE = {}


def kernel(**inputs):
    N = int(np.asarray(inputs["node_embeddings"]).shape[0])
    n_cores = 8
    g = Geo(N=N, n_cores=n_cores)

    sched, in_maps = host_prep(g, **{k: np.asarray(v) for k, v in inputs.items()})

    key = (N, sched["total_chunks"], tuple(int(x) for x in sched["caps"].ravel()),
           sched["skip_norm"])
    if key not in _CACHE:
        _CACHE[key] = build_program(g, sched)
    nc = _CACHE[key]

    from concourse.bass_utils import run_bass_kernel_spmd
    res = run_bass_kernel_spmd(nc, in_maps, core_ids=list(range(n_cores)))
    # un-permute the window-balanced rows (device row pos[i] holds node i)
    out = np.concatenate([res.results[c]["out"][sched["poss"][c]]
                          for c in range(n_cores)], axis=0)
    return out.astype(np.float32)


# revision 34
# speedup vs baseline: 1.1664x; 1.1601x over previous
"""GATv2 graph layer Bass kernel for TRN2 (SPMD across 8 NeuronCores, no
collectives).

Design (v3): edges sorted by destination node and sharded across cores by dst
range. Each core builds ONE fp16 gather table in DRAM (xsrc = node_emb @
W_src, split in two halves so int16 gather indices fit), with rows remapped so
table writes use 1KB descriptors. Destination-side features are NOT gathered
from DRAM: x_dst for the core's own nodes lives in SBUF ([P, nw, HID] f16),
and per edge chunk (128 edges, one 128-node dst window) the dst contribution
is gathered by the TensorEngine via a transposed one-hot:

  oh[e, d]  = (dstr[e] == d)                       (DVE tensor_scalar)
  ohT[d, e] = transpose(oh)                        (PE transpose, f16 PSUM)
  psum_combT[hid, e] = xdst_win^T @ ohT            (lhsT=xdst_win, run-batched)
                     + emb8^T @ ohetT              (one matmul per group)
                     + xs_chunk^T                  (lhsT=xs, rhs=identity)
  combT16 = Prelu(psum_combT, alpha=0.2)           (Act)
  ex_ps[e, h] = combT16^T @ att_blk                (PE)
  ex16 = Exp(ex_ps)                                (Act, per group)
  wgt = xs * bcast(ex16)                           (DVE, 4 chunks per op)
  win_ps[dst, 4+HID] += oh^T @ [ex16 | wgt]        (PE scatter)

The hidden dim is stored d-major ((d,h) instead of (h,d), a host-side column
permutation of W_src/W_dst/emb/att/W_out) so the per-head broadcast of ex in
the weight-mul has a packed innermost dim — that enables the DVE 2x perf mode
and 4-chunk batching. FiLM (tanh) and the W_out column scaling are folded
into constants on the host, so the only device activation functions are
{Prelu, Exp, Ln} — all within one activation table (no reload thrash; the
LayerNorm rstd is exp(-0.5*ln(var+eps))).

Window flush (batched across the slab's windows): agg = sum(ex*xs)/sum(ex),
@Wosc + beta_eff, residual, LayerNorm via bn_stats/bn_aggr, f16 DMA out.
No max-subtraction in the softmax: logits are bounded so exp stays finite in
f32 (matches reference to ~1e-4).
"""
import numpy as np
from contextlib import ExitStack
from dataclasses import dataclass

import concourse.bass as bass
import concourse.tile as tile
from concourse import bacc, mybir
from concourse.masks import make_identity

P = 128
HID = 128
H = 4
HD = 32
NET = 8
EPS_LN = 1e-5
MAXCALL = 4096  # max idxs per dma_gather call
DEAD = -5.0     # dst_rel for padding slots (matches no one-hot column)


@dataclass
class Geo:
    N: int
    n_cores: int
    slab_w: int = 3     # windows per slab
    lookahead: int = 2  # slabs of input prefetch
    ohT_act_mod: int = 8    # every k-th group's ohT copy runs on Act
    oh_pool_mod: int = 4    # every k-th chunk's oh gen runs on Pool
    wmul_pool_mod: int = 3  # every k-th chunk's weight-mul runs on Pool

    @property
    def npc(self):
        return self.N // self.n_cores

    @property
    def nw(self):
        return (self.npc + P - 1) // P

    @property
    def nslab(self):
        return (self.nw + self.slab_w - 1) // self.slab_w

    @property
    def n_pad(self):   # padded node count (512 blocks)
        return ((self.N + 511) // 512) * 512

    @property
    def split(self):   # lo/hi table split on a 512 block boundary
        return (self.n_pad // 1024) * 512


def wrap_idx(idx, cols):
    n = idx.shape[0]
    assert n % 16 == 0
    w = np.zeros((P, cols), dtype=np.int16)
    if n:
        t16 = idx.reshape(n // 16, 16).T
        for g in range(8):
            w[g * 16:(g + 1) * 16, :n // 16] = t16
    return w


def remap_row(r):
    """Table row remap so device-side table writes are 1KB/partition:
    original row i*512 + s*128 + p is stored at i*512 + p*4 + s."""
    i, rem = r // 512, r % 512
    return i * 512 + (rem % 128) * 4 + rem // 128


def host_prep(g: Geo, node_embeddings, edge_index, edge_type, task_embedding,
              W_src, b_src, W_dst, b_dst, edge_emb, att,
              W_out, b_out, norm_w, norm_b, W_film, b_film):
    """Returns (sched, in_maps). Pure index work + tiny constant folding;
    all O(N*HID) / O(E*HID) float math runs on device."""
    src = np.asarray(edge_index[0], dtype=np.int64)
    dst = np.asarray(edge_index[1], dtype=np.int64)
    et = np.asarray(edge_type, dtype=np.int64)
    npc, split = g.npc, g.split

    order = np.argsort(dst, kind="stable")
    src, dst, et = src[order], dst[order], et[order]
    core_of = dst // npc

    # Per-core window balancing: permute each core's own nodes across its
    # windows so per-(window, half) edge counts are near-equal across all
    # cores. caps[w,h] is a max over cores, so imbalance pads every core's
    # chunk schedule; balancing cuts total chunks ~15% (pure host index
    # work; the output rows are un-permuted on the host afterwards).
    w_caps = np.array([min(P, npc - w * P) for w in range(g.nw)])
    poss, invs = [], []
    buckets = {}
    for c in range(g.n_cores):
        m = core_of == c
        cs, cd, ce = src[m], dst[m] - c * npc, et[m]
        lo_m = cs < split
        lo_deg = np.bincount(cd[lo_m], minlength=npc).astype(np.float64)
        hi_deg = np.bincount(cd[~lo_m], minlength=npc).astype(np.float64)
        order = np.argsort(-(lo_deg + hi_deg), kind="stable")
        Llo = np.zeros(g.nw)
        Lhi = np.zeros(g.nw)
        nfill = np.zeros(g.nw, dtype=np.int64)
        pos = np.empty(npc, dtype=np.int64)
        # objective = chunk count: stay within 5*P edges per half where
        # possible; unavoidable overflow chunks concentrate in the highest
        # windows (same ones on every core -> shared caps stay tight)
        TGT = 5 * P
        wrank = (g.nw - np.arange(g.nw)) * 1e6
        for node in order:
            costl = Llo + lo_deg[node]
            costh = Lhi + hi_deg[node]
            extra = (np.ceil(np.maximum(costl, TGT) / P) - 5
                     + np.ceil(np.maximum(costh, TGT) / P) - 5)
            key = extra * 1e9 + np.where(extra > 0, wrank, 0.0) \
                + np.maximum(costl, costh)
            key[nfill >= w_caps] = np.inf
            w = int(np.argmin(key))
            pos[node] = w * P + nfill[w]
            nfill[w] += 1
            Llo[w] += lo_deg[node]
            Lhi[w] += hi_deg[node]
        # swap-repair: the greedy's endgame is slot-capacity-bound, leaving
        # a few windows a couple of edges over TGT; pairwise node swaps with
        # slack windows remove (or concentrate in the last windows) the
        # overflow so the shared caps stay at 5 chunks per half
        members = [[] for _ in range(g.nw)]
        for node in range(npc):
            members[pos[node] // P].append(node)
        for _ in range(400):
            over = np.maximum(Llo - TGT, 0) + np.maximum(Lhi - TGT, 0)
            over[g.nw - 1] = 0.0   # tail window is the designated spill
            wb = int(np.argmax(over))
            if over[wb] <= 0:
                break
            def do_swap(a, b, u):
                members[wb].remove(a)
                members[u].remove(b)
                members[wb].append(b)
                members[u].append(a)
                pos[a], pos[b] = pos[b], pos[a]
                Llo[wb] += lo_deg[b] - lo_deg[a]
                Lhi[wb] += hi_deg[b] - hi_deg[a]
                Llo[u] += lo_deg[a] - lo_deg[b]
                Lhi[u] += hi_deg[a] - hi_deg[b]

            done = False
            cand_a = sorted(members[wb],
                            key=lambda n: -(lo_deg[n] + hi_deg[n]))[:12]
            for u in np.argsort(np.maximum(Llo, Lhi)):
                if u == wb:
                    continue
                for a in cand_a:
                    la, ha = lo_deg[a], hi_deg[a]
                    bs = members[u]
                    lb = lo_deg[bs]
                    hb = hi_deg[bs]
                    ok = ((Llo[wb] - la + lb <= TGT)
                          & (Lhi[wb] - ha + hb <= TGT)
                          & (Llo[u] + la - lb <= TGT)
                          & (Lhi[u] + ha - hb <= TGT))
                    if ok.any():
                        do_swap(a, bs[int(np.argmax(ok))], u)
                        done = True
                        break
                if done:
                    break
            if not done:
                # totals force overflow somewhere: concentrate it in the
                # tail window (same window on every core)
                u = g.nw - 1
                a = cand_a[0]
                bs = members[u]
                j = int(np.argmin(lo_deg[bs] + hi_deg[bs]))
                b = bs[j]
                if lo_deg[a] + hi_deg[a] > lo_deg[b] + hi_deg[b]:
                    do_swap(a, b, u)
                else:
                    break
        # only the last window is short, so slots coincide with compact rows
        assert pos.max() < npc
        inv = np.empty(npc, dtype=np.int64)
        inv[pos] = np.arange(npc)    # new row -> old local row
        poss.append(pos)
        invs.append(inv)

        cdp = pos[cd]                # edge dst -> new row (window-aligned)
        for w in range(g.nw):
            wm = (cdp // P) == w
            ws_, wd, we = cs[wm], cdp[wm] - w * P, ce[wm]
            lo = ws_ < split
            buckets[(c, w, 0)] = (remap_row(ws_[lo]), wd[lo], we[lo])
            buckets[(c, w, 1)] = (remap_row(ws_[~lo] - split), wd[~lo], we[~lo])

    caps = np.zeros((g.nw, 2), dtype=np.int64)
    for w in range(g.nw):
        for h in range(2):
            mx = max(len(buckets[(c, w, h)][0]) for c in range(g.n_cores))
            caps[w, h] = (mx + P - 1) // P

    # ---- schedule ---------------------------------------------------------
    sched_slabs = []
    total_chunks = 0
    for s in range(g.nslab):
        ws = list(range(s * g.slab_w, min((s + 1) * g.slab_w, g.nw)))
        chunks = []            # (win_local, half, slot)
        calls = {0: [], 1: []}
        slot = 0
        for h in (0, 1):
            run = 0
            run_start = slot
            for w in ws:
                for _ in range(caps[w, h]):
                    chunks.append((w - ws[0], h, slot))
                    slot += 1
                    run += P
                    if run == MAXCALL:
                        calls[h].append((run_start, run))
                        run, run_start = 0, slot
            if run:
                calls[h].append((run_start, run))
        sched_slabs.append(dict(windows=ws, chunks=chunks, calls=calls,
                                chunk0=total_chunks))
        total_chunks += len(chunks)

    lo_cols = max(16, sum(n for sl in sched_slabs
                          for (_, n) in sl["calls"][0]) // 16)
    hi_cols = max(16, sum(n for sl in sched_slabs
                          for (_, n) in sl["calls"][1]) // 16)

    # ---- shared constants -------------------------------------------------
    # d-major permutation of the hidden dim: new col j holds old col
    # (j%H)*HD + j//H, i.e. feature (h, d) lives at j = d*H + h.
    perm = np.array([(j % H) * HD + j // H for j in range(HID)], dtype=np.int64)

    nodeT = np.zeros((HID, g.n_pad), dtype=np.float16)
    nodeT[:, :g.N] = np.asarray(node_embeddings, np.float32).T.astype(np.float16)
    emb_eff = (np.asarray(edge_emb, np.float64)
               + np.asarray(b_src, np.float64)[None, :]
               + np.asarray(b_dst, np.float64)[None, :])[:, perm].astype(np.float16)
    att_blk = np.zeros((HID, H), dtype=np.float16)
    att64 = np.asarray(att, np.float64)
    for h in range(H):
        for d in range(HD):
            att_blk[d * H + h, h] = att64[h, d]

    # FiLM fold (O(HID^2) host math): gamma/beta modulation of the output
    # projection becomes a column scale on W_out plus a bias.
    film = (np.asarray(task_embedding, np.float64)
            @ np.asarray(W_film, np.float64)
            + np.asarray(b_film, np.float64))
    gm = 1.0 + 0.5 * np.tanh(film[:HID])
    beta_eff = np.asarray(b_out, np.float64) * gm + film[HID:]
    Wosc = (np.asarray(W_out, np.float64)[perm, :] * gm[None, :])

    consts = dict(
        nodeT=nodeT,
        W_src=np.asarray(W_src, np.float32)[:, perm].astype(np.float16),
        W_dst=np.asarray(W_dst, np.float32)[:, perm].astype(np.float16),
        Wosc=Wosc.astype(np.float16),
        beta=beta_eff.astype(np.float16).reshape(1, HID),
        emb8=emb_eff,                       # [8, HID] f16 (permuted)
        att_blk=att_blk,
    )
    skip_norm = bool(np.all(np.asarray(norm_w) == 1.0)
                     and np.all(np.asarray(norm_b) == 0.0))
    if not skip_norm:
        consts["normw"] = np.asarray(norm_w, np.float32).reshape(1, HID)
        consts["normb"] = np.asarray(norm_b, np.float32).reshape(1, HID)

    node_f16 = np.asarray(node_embeddings, np.float32).astype(np.float16)

    # ---- per-core arrays --------------------------------------------------
    in_maps = []
    for c in range(g.n_cores):
        lo_l, hi_l = [], []
        dstr = np.full((P, total_chunks), DEAD, dtype=np.float32)
        oet = np.zeros((NET, total_chunks * P), dtype=np.float16)
        ci = 0
        for sl in sched_slabs:
            ws0 = sl["windows"][0]
            per_half = {0: [], 1: []}
            nth = {}
            for (wl, h, slot) in sl["chunks"]:
                w = ws0 + wl
                es, ed, ee = buckets[(c, w, h)]
                k = nth.get((wl, h), 0)
                nth[(wl, h)] = k + 1
                sl_src = np.zeros(P, dtype=np.int64)
                n = min(P, max(0, len(es) - k * P))
                if n > 0:
                    sl_src[:n] = es[k * P:k * P + n]
                    dstr[:n, ci] = ed[k * P:k * P + n]
                    oet[ee[k * P:k * P + n], ci * P + np.arange(n)] = 1.0
                per_half[h].append(sl_src)
                ci += 1
            lo_l.extend(per_half[0])
            hi_l.extend(per_half[1])
        lo_i = (np.concatenate(lo_l) if lo_l else np.zeros(0, np.int64))
        hi_i = (np.concatenate(hi_l) if hi_l else np.zeros(0, np.int64))
        assert lo_i.max(initial=0) < g.split <= 32767
        assert hi_i.max(initial=0) < g.n_pad - g.split <= 32768

        own = node_f16[c * npc:(c + 1) * npc][invs[c]]  # [npc, HID] permuted
        ownT = np.zeros((HID, g.nw * P), dtype=np.float16)
        ownT[:, :npc] = own.T

        m = dict(consts)
        m["node_own16"] = np.ascontiguousarray(own)
        m["node_ownT"] = ownT
        m["lo_idx"] = wrap_idx(lo_i.astype(np.int16), lo_cols)
        m["hi_idx"] = wrap_idx(hi_i.astype(np.int16), hi_cols)
        m["dstr"] = dstr
        m["ohetT"] = oet
        in_maps.append(m)

    sched = dict(slabs=sched_slabs, caps=caps, total_chunks=total_chunks,
                 lo_cols=lo_cols, hi_cols=hi_cols, skip_norm=skip_norm,
                 poss=poss)
    return sched, in_maps


def build_program(g: Geo, sched, debug=False):
    nc = bacc.Bacc("TRN2", target_bir_lowering=False, debug=False,
                   num_devices=g.n_cores, num_swdge_queues=4)
    f16, f32 = mybir.dt.float16, mybir.dt.float32
    AF = mybir.ActivationFunctionType
    OP = mybir.AluOpType
    npc, nw = g.npc, g.nw
    total_chunks = sched["total_chunks"]
    lo_cols, hi_cols = sched["lo_cols"], sched["hi_cols"]

    def din(name, shape, dt):
        return nc.dram_tensor(name, shape, dt, kind="ExternalInput").ap()

    nodeT = din("nodeT", [HID, g.n_pad], f16)
    node_own16 = din("node_own16", [npc, HID], f16)
    node_ownT = din("node_ownT", [HID, nw * P], f16)
    W_src = din("W_src", [HID, HID], f16)
    W_dst = din("W_dst", [HID, HID], f16)
    Wosc_dr = din("Wosc", [HID, HID], f16)
    beta_dr = din("beta", [1, HID], f16)
    emb8 = din("emb8", [NET, HID], f16)
    att_blk = din("att_blk", [HID, H], f16)
    lo_idx = din("lo_idx", [P, lo_cols], mybir.dt.int16)
    hi_idx = din("hi_idx", [P, hi_cols], mybir.dt.int16)
    dstr = din("dstr", [P, total_chunks], f32)
    ohetT = din("ohetT", [NET, total_chunks * P], f16)
    out = nc.dram_tensor("out", [npc, HID], f16, kind="ExternalOutput").ap()

    xsrc_tab = nc.dram_tensor("xsrc_tab", [g.n_pad, HID], f16,
                              kind="Internal").ap()

    with tile.TileContext(nc, trace_sim=False) as tc, ExitStack() as ctx:
        cpool = ctx.enter_context(tc.tile_pool(name="consts", bufs=1))
        bpool = ctx.enter_context(tc.tile_pool(name="build", bufs=3))
        # PSUM: 8 banks of 2KB/partition, one per tile buf. Exactly 8 bufs.
        psBig = ctx.enter_context(tc.tile_pool(name="psBig", bufs=2,
                                               space="PSUM"))
        psT4 = ctx.enter_context(tc.tile_pool(name="psT4", bufs=2,
                                              space="PSUM"))
        psWin = ctx.enter_context(tc.tile_pool(name="psWin", bufs=2,
                                               space="PSUM"))
        psEx = ctx.enter_context(tc.tile_pool(name="psEx", bufs=1,
                                              space="PSUM"))
        psSm = ctx.enter_context(tc.tile_pool(name="psSm", bufs=1,
                                              space="PSUM"))
        spool = ctx.enter_context(tc.tile_pool(name="slab",
                                               bufs=g.lookahead + 1))
        gpool = ctx.enter_context(tc.tile_pool(name="grp", bufs=3))
        fpool = ctx.enter_context(tc.tile_pool(name="flush", bufs=2))

        # ---- idx + dstr staging (first: gathers only wait on the table) ---
        lo_sb = cpool.tile([P, lo_cols], mybir.dt.int16, tag="loidx")
        nc.sync.dma_start(lo_sb[:], lo_idx[:])
        hi_sb = cpool.tile([P, hi_cols], mybir.dt.int16, tag="hiidx")
        nc.sync.dma_start(hi_sb[:], hi_idx[:])
        dstr_sb = cpool.tile([P, total_chunks], f32, tag="dstr")
        nc.sync.dma_start(dstr_sb[:], dstr[:])

        # ---- constants ----------------------------------------------------
        ident = cpool.tile([P, P], f16)
        make_identity(nc, ident[:])
        iota16 = cpool.tile([P, P], mybir.dt.int16)
        nc.gpsimd.iota(iota16[:], pattern=[[1, P]], base=0, channel_multiplier=0)
        iota = cpool.tile([P, P], f16)
        nc.vector.tensor_copy(iota[:], iota16[:])
        ones_row = cpool.tile([1, P], f16)
        nc.vector.memset(ones_row[:], 1.0)
        eps_col = cpool.tile([P, 1], f32)
        nc.vector.memset(eps_col[:], EPS_LN)

        Ws = cpool.tile([HID, HID], f16)
        nc.sync.dma_start(Ws[:], W_src[:])
        Wd = cpool.tile([HID, HID], f16)
        nc.sync.dma_start(Wd[:], W_dst[:])
        Wosc = cpool.tile([HID, HID], f16)
        nc.sync.dma_start(Wosc[:], Wosc_dr[:])
        beta16 = cpool.tile([1, HID], f16)
        nc.sync.dma_start(beta16[:], beta_dr[:])
        emb_sb = cpool.tile([NET, HID], f16)
        nc.sync.dma_start(emb_sb[:], emb8[:])
        att_sb = cpool.tile([HID, H], f16)
        nc.sync.dma_start(att_sb[:], att_blk[:])

        # residual (own node embeddings) as [P, nw, HID] f16
        resid16 = cpool.tile([P, nw, HID], f16, tag="resid")
        tail = npc - (npc // P) * P
        full_w = npc // P
        if tail:
            nc.vector.memset(resid16[:, full_w, :], 0.0)
        if full_w:
            nc.sync.dma_start(
                resid16[:, :full_w, :],
                node_own16[:full_w * P, :].rearrange("(w p) h -> p w h", p=P))
        if tail:
            nc.sync.dma_start(resid16[:tail, full_w, :],
                              node_own16[full_w * P:, :])

        if not sched["skip_norm"]:
            nw_dr = din("normw", [1, HID], f32)
            nb_dr = din("normb", [1, HID], f32)
            nw_sb = cpool.tile([1, HID], f32)
            nc.sync.dma_start(nw_sb[:], nw_dr[:])
            nb_sb = cpool.tile([1, HID], f32)
            nc.sync.dma_start(nb_sb[:], nb_dr[:])
            ones32 = cpool.tile([1, P], f32)
            nc.vector.memset(ones32[:], 1.0)
            ps_w = psBig.tile([P, HID], f32, space="PSUM", tag="pcb")
            nc.tensor.matmul(out=ps_w[:], lhsT=ones32[:], rhs=nw_sb[:],
                             start=True, stop=True)
            w_rep = cpool.tile([P, HID], f32)
            nc.vector.tensor_copy(w_rep[:], ps_w[:])
            ps_b = psBig.tile([P, HID], f32, space="PSUM", tag="pcb")
            nc.tensor.matmul(out=ps_b[:], lhsT=ones32[:], rhs=nb_sb[:],
                             start=True, stop=True)
            b_rep = cpool.tile([P, HID], f32)
            nc.vector.tensor_copy(b_rep[:], ps_b[:])

        # ---- xdst table (SBUF-resident) -----------------------------------
        xdst_sb = cpool.tile([P, nw, HID], f16, tag="xdst")
        for gl in range(0, nw, 16):
            gln = min(16, nw - gl)
            not_t = bpool.tile([HID, 16 * P], f16, tag="not")
            nc.sync.dma_start(not_t[:, :gln * P],
                              node_ownT[:, gl * P:(gl + gln) * P])
            for gw in range(gl, gl + gln, 4):
                gn = min(4, gl + gln - gw)
                psX = psBig.tile([P, 4, HID], f32, space="PSUM", tag="pcb")
                for k in range(gn):
                    nc.tensor.matmul(out=psX[:, k, :],
                                     lhsT=not_t[:, (gw - gl + k) * P:
                                                (gw - gl + k + 1) * P],
                                     rhs=Wd[:],
                                     start=True, stop=True,
                                     skip_group_check=True)
                nc.scalar.activation(xdst_sb[:, gw:gw + gn, :], psX[:, :gn, :],
                                     AF.Prelu, alpha=1.0)

        # ---- xsrc gather table --------------------------------------------
        # 2048-node iterations: one big nodeT load (SP) + one big table write
        # (Act) each -- HWDGE charges ~0.6us per DMA instruction, so few big
        # DMAs beat many small ones. PSUM->SBUF copies alternate DVE/Act.
        starts = list(range(0, g.n_pad, 2048))  # last block may be partial
        nts = {}

        def load_nt(i):
            st = starts[i]
            wdt = min(2048, g.n_pad - st)
            nt = bpool.tile([HID, 2048], f16, tag="nt", name=f"nt{i}",
                            bufs=3)
            nc.sync.dma_start(nt[:, :wdt], nodeT[:, st:st + wdt])
            nts[i] = nt

        # interleave loads and writes (prefetch 2) so the DMA device
        # alternates them: the lo table half finishes ~18us earlier and the
        # first slabs' lo gathers start correspondingly sooner
        load_nt(0)
        load_nt(1)
        for i, st in enumerate(starts):
            if i + 2 < len(starts):
                load_nt(i + 2)
            wdt = min(2048, g.n_pad - st)
            nq = wdt // 512
            nt = nts.pop(i)
            xt = bpool.tile([P, 4, 4, HID], f16, tag="xt", bufs=3)
            for q in range(nq):
                ps = psBig.tile([P, 4, HID], f32, space="PSUM", tag="pcb")
                for j in range(4):
                    nc.tensor.matmul(out=ps[:, j, :],
                                     lhsT=nt[:, q * 512 + j * P:
                                             q * 512 + (j + 1) * P],
                                     rhs=Ws[:],
                                     start=True, stop=True,
                                     skip_group_check=True)
                if q % 2 == 0:
                    nc.vector.tensor_copy(xt[:, q, :, :], ps[:])
                else:
                    nc.scalar.activation(xt[:, q, :, :], ps[:],
                                         AF.Prelu, alpha=1.0)
            # dram row st + q*512 + p*4 + s  <-  xt[p, q, s, :]
            nc.scalar.dma_start(
                xsrc_tab[st:st + wdt, :]
                .rearrange("(q p s) h -> p q s h", p=P, s=4),
                xt[:, :nq, :, :])

        off16 = {0: 0, 1: 0}
        qn = {0: 0, 1: 0}

        def nextq(h):
            # dedicated queues per table half: a hi gather parked on the full
            # table must not block later lo gathers (lo half is ready first)
            qn[h] ^= 1
            return 2 * h + qn[h]

        # ---- edge slabs ----------------------------------------------------
        # Software-pipelined emission: each slab's input loads and one-hot
        # generation ("front") are emitted `lookahead` slabs ahead of its
        # compute + flush ("back"), so the in-order per-engine sequencers
        # never park a next-slab load behind a previous slab's dependency
        # waits.
        fronts = {}
        wins = {}
        pending_scat = {}

        def emit_front(s):
            sl = sched["slabs"][s]
            chunks = sl["chunks"]
            C = len(chunks)
            c0 = sl["chunk0"]

            xs_t = spool.tile([P, C, HID], f16, tag="xs", name=f"xs{s}")
            for h in (0, 1):
                base = 0 if h == 0 else g.split
                lim = g.split if h == 0 else g.n_pad
                idx_sb = lo_sb if h == 0 else hi_sb
                for (slot_off, n) in sl["calls"][h]:
                    if n == 0:
                        continue
                    nc.gpsimd.dma_gather(
                        out_ap=xs_t[:, slot_off:slot_off + n // P, :],
                        in_ap=xsrc_tab[base:lim, :],
                        idxs_ap=idx_sb[:, off16[h]:off16[h] + n // 16],
                        num_idxs=n, num_idxs_reg=n, elem_size=HID,
                        single_packet=(n <= 1024), queue_num=nextq(h),
                    )
                    off16[h] += n // 16

            oet_t = spool.tile([NET, C * P], f16, tag="oet", name=f"oet{s}")
            nc.sync.dma_start(oet_t[:], ohetT[:, c0 * P:(c0 + C) * P])

            oh_t = spool.tile([P, C, P], f16, tag="oh", name=f"oh{s}",
                              bufs=g.lookahead + 2)
            for ci in range(C):
                eng = (nc.gpsimd if ci % g.oh_pool_mod == g.oh_pool_mod - 1
                       else nc.vector)
                eng.tensor_scalar(
                    oh_t[:, ci, :], iota[:], dstr_sb[:, c0 + ci:c0 + ci + 1],
                    None, OP.is_equal)
            fronts[s] = (xs_t, oet_t, oh_t)

        def emit_back(s):
            sl = sched["slabs"][s]
            ws = sl["windows"]
            nwin = len(ws)
            chunks = sl["chunks"]
            C = len(chunks)
            c0 = sl["chunk0"]
            xs_t, oet_t, oh_t = fronts.pop(s)

            ohT_t = spool.tile([P, C, P], f16, tag="ohT", name=f"ohT{s}",
                               bufs=2)
            rhs_t = spool.tile([P, C, 4 + HID], f16, tag="rhs",
                               name=f"rhs{s}", bufs=2)
            ex_ps = psEx.tile([P, C, H], f32, space="PSUM", tag="ex")

            # Group-level software pipeline (stagger 1): the PE sequencer is
            # in-order and the cost model's DVFS ramp punishes every PE stall,
            # so each PE stage must only consume results produced >= 1 group
            # earlier on the other engines.
            ngrp = (C + 3) // 4

            def g_n(gi):
                return min(4, C - 4 * gi)

            pcbs = {}

            psT_pair = {}

            def stage_transpose(gi):          # PE <- oh (front, ready)
                gn, cs0 = g_n(gi), 4 * gi
                # one PSUM bank holds two groups' transposes ([P, 8, P] f16),
                # so 2 bank-granular buffers give a 4-group WAR horizon
                if gi % 2 == 0:
                    psT_pair[gi // 2] = psT4.tile([P, 8, P], f16,
                                                  space="PSUM", tag="psT",
                                                  name=f"psT{s}_{gi // 2}")
                psT = psT_pair[gi // 2][:, (gi % 2) * 4:(gi % 2) * 4 + 4, :]
                for k in range(gn):
                    nc.tensor.transpose(out=psT[:, k, :],
                                        in_=oh_t[:, cs0 + k, :],
                                        identity=ident[:])
                return psT

            def stage_copy(gi, psT):          # DVE/Act <- psT
                gn, cs0 = g_n(gi), 4 * gi
                if gi % g.ohT_act_mod == g.ohT_act_mod - 1:
                    nc.scalar.activation(ohT_t[:, cs0:cs0 + gn, :],
                                         psT[:, :gn, :], AF.Prelu, alpha=1.0)
                else:
                    nc.vector.tensor_copy(ohT_t[:, cs0:cs0 + gn, :],
                                          psT[:, :gn, :])

            def stage_pcb(gi):                # PE <- ohT copy (1 group ago)
                gn, cs0 = g_n(gi), 4 * gi
                pcb = psBig.tile([HID, 4, P], f32, space="PSUM", tag="pcb",
                                 name=f"pcb{s}_{gi}")
                for k in range(gn):
                    ci = cs0 + k
                    wl = chunks[ci][0]
                    nc.tensor.matmul(out=pcb[:, k, :],
                                     lhsT=xdst_sb[:, ws[wl], :],
                                     rhs=ohT_t[:, ci, :],
                                     start=True, stop=False,
                                     skip_group_check=True)
                    nc.tensor.matmul(out=pcb[:, k, :], lhsT=emb_sb[:],
                                     rhs=oet_t[:, ci * P:(ci + 1) * P],
                                     start=False, stop=False,
                                     skip_group_check=True)
                    nc.tensor.matmul(out=pcb[:, k, :], lhsT=xs_t[:, ci, :],
                                     rhs=ident[:], start=False, stop=True,
                                     skip_group_check=True)
                pcbs[gi] = pcb

            def stage_prelu(gi):              # Act <- pcb
                gn, cs0 = g_n(gi), 4 * gi
                combT = gpool.tile([HID, 4, P], f16, tag="comb",
                                   name=f"cmb{s}_{gi}")
                nc.scalar.activation(combT[:, :gn, :], pcbs.pop(gi)[:, :gn, :],
                                     AF.Prelu, alpha=0.2)
                return combT

            combTs = {}

            def stage_att(gi):                # PE <- combT (1 group ago)
                gn, cs0 = g_n(gi), 4 * gi
                combT = combTs.pop(gi)
                for k in range(gn):
                    ci = cs0 + k
                    nc.tensor.matmul(out=ex_ps[:, ci, :], lhsT=combT[:, k, :],
                                     rhs=att_sb[:], start=True, stop=True,
                                     skip_group_check=True)

            def stage_exp(gi):                # Act <- ex_ps
                gn, cs0 = g_n(gi), 4 * gi
                nc.scalar.activation(rhs_t[:, cs0:cs0 + gn, 0:4],
                                     ex_ps[:, cs0:cs0 + gn, :], AF.Exp)

            def stage_wmul(gi):               # DVE/Pool <- exp, xs
                gn, cs0 = g_n(gi), 4 * gi
                for k in range(gn):
                    ci = cs0 + k
                    eng = (nc.gpsimd if (ci % g.wmul_pool_mod
                                         == g.wmul_pool_mod - 1)
                           else nc.vector)
                    eng.tensor_mul(
                        rhs_t[:, ci, 4:].rearrange("p (d h) -> p d h", d=HD),
                        xs_t[:, ci, :].rearrange("p (d h) -> p d h", d=HD),
                        rhs_t[:, ci, 0:4].unsqueeze(1)
                        .broadcast_to([P, HD, H]))

            # previous slab's deferred scatter ops: their weight-muls are
            # long done, so interleaving them here never parks the PE queue
            prev_scat = pending_scat.pop(s - 1, [])
            niter = ngrp + 3
            per_iter = -(-len(prev_scat) // niter) if prev_scat else 0
            sci = 0

            for gi in range(niter):
                if gi < ngrp:
                    stage_copy(gi, stage_transpose(gi))
                if 2 <= gi < ngrp + 2:
                    stage_pcb(gi - 2)
                    combTs[gi - 2] = stage_prelu(gi - 2)
                if 3 <= gi:
                    stage_att(gi - 3)
                    stage_exp(gi - 3)
                    stage_wmul(gi - 3)
                for _ in range(per_iter):
                    if sci < len(prev_scat):
                        prev_scat[sci]()
                        sci += 1
            while sci < len(prev_scat):
                prev_scat[sci]()
                sci += 1

            # -- deferred scatter into this slab's window accumulators
            win_t = psWin.tile([P, g.slab_w, 4 + HID], f32, space="PSUM",
                               tag="win", name=f"win{s}")
            n_per_win = [0] * nwin
            for (wl, h, slot) in chunks:
                n_per_win[wl] += 1
            # window-major scatter order: accumulation groups sharing a
            # PSUM bank must not interleave on hardware
            seen = [0] * nwin
            order = sorted(range(C), key=lambda ci: chunks[ci][0])
            scat = []
            for ci in order:
                wl = chunks[ci][0]
                first = seen[wl] == 0
                last = seen[wl] == n_per_win[wl] - 1
                seen[wl] += 1
                scat.append(lambda ci=ci, wl=wl, first=first, last=last:
                            nc.tensor.matmul(out=win_t[:, wl, :],
                                             lhsT=oh_t[:, ci, :],
                                             rhs=rhs_t[:, ci, :],
                                             start=first, stop=last,
                                             skip_group_check=True))
            pending_scat[s] = scat
            wins[s] = (win_t, n_per_win)

        def emit_flush(s):
            # ---- flush windows (emitted one slab late so these dependency-
            # waiting ops never park in front of the next slab's work on the
            # in-order per-engine sequencers). Elementwise work is batched
            # across the slab's windows. -------------------------------------
            sl = sched["slabs"][s]
            ws = sl["windows"]
            nwin = len(ws)
            win_t, n_per_win = wins.pop(s)
            assert all(n > 0 for n in n_per_win)
            # f32: raw exp-sums can exceed the f16 max before the
            # normalization divide
            win16 = fpool.tile([P, nwin, 4 + HID], f32, tag="win16",
                               name=f"w16_{s}")
            nc.scalar.activation(win16[:], win_t[:, :nwin, :], AF.Prelu,
                                 alpha=1.0)
            sums = fpool.tile([P, nwin, 4], f32, tag="sums")
            nc.vector.tensor_scalar(sums[:], win16[:, :, 0:4], 1e-12, None,
                                    OP.max)
            rec = fpool.tile([P, nwin, 4], f32, tag="rec")
            nc.vector.reciprocal(rec[:], sums[:])
            aggn = fpool.tile([P, nwin, HID], f16, tag="aggn")
            nc.vector.tensor_mul(
                aggn[:].rearrange("p w (d h) -> p w d h", d=HD),
                win16[:, :, 4:].rearrange("p w (d h) -> p w d h", d=HD),
                rec[:].unsqueeze(2).broadcast_to([P, nwin, HD, H]))
            psTf = psT4.tile([P, 8, P], f16, space="PSUM", tag="psT")
            for wl in range(nwin):
                nc.tensor.transpose(out=psTf[:, wl, :], in_=aggn[:, wl, :],
                                    identity=ident[:])
            aggT = fpool.tile([HID, nwin, P], f16, tag="aggT")
            nc.vector.tensor_copy(aggT[:], psTf[:, :nwin, :])
            po = psSm.tile([P, g.slab_w, HID], f32, space="PSUM", tag="pt")
            for wl in range(nwin):
                nc.tensor.matmul(out=po[:, wl, :], lhsT=aggT[:, wl, :],
                                 rhs=Wosc[:], start=True, stop=False,
                                 skip_group_check=True)
                nc.tensor.matmul(out=po[:, wl, :], lhsT=ones_row[:],
                                 rhs=beta16[:], start=False, stop=True,
                                 skip_group_check=True)
            y = fpool.tile([P, nwin, HID], f32, tag="y", name=f"y{s}")
            nc.vector.tensor_add(y[:], po[:, :nwin, :],
                                 resid16[:, ws[0]:ws[0] + nwin, :])
            bn_sl = fpool.tile([P, nwin, 2], f32, tag="bnsl", name=f"bns{s}")
            for wl in range(nwin):
                bnst = fpool.tile([P, 6], f32, tag="bnst")
                nc.vector.bn_stats(bnst[:], y[:, wl, :])
                nc.vector.bn_aggr(bn_sl[:, wl, :], bnst[:])
            # rstd = exp(-0.5 * ln(var + eps)): keeps the Act engine within
            # one activation-function table ({Prelu, Exp, Ln}; Sqrt would
            # force a table reload every slab)
            # rstd = 1/sqrt(var+eps) fully on DVE (quake seed + 3 Newton
            # steps, ~2e-7): Sqrt/Ln on Act would force an activation-table
            # reload every slab (only {Prelu, Exp} stay loaded).
            vr = fpool.tile([P, nwin], f32, tag="vr", name=f"vr{s}")
            nc.vector.tensor_scalar(vr[:], bn_sl[:, :, 1], EPS_LN, None,
                                    OP.add)
            qi = fpool.tile([P, nwin], mybir.dt.int32, tag="qi",
                            name=f"qi{s}")
            nc.vector.tensor_scalar(qi[:], vr[:].bitcast(mybir.dt.int32),
                                    1, None, OP.logical_shift_right)
            nc.vector.tensor_scalar(qi[:], qi[:], -1, 0x5f3759df,
                                    OP.mult, OP.add)
            rstd = fpool.tile([P, nwin], f32, tag="rstd", name=f"rs{s}")
            nc.vector.tensor_copy(rstd[:].bitcast(mybir.dt.int32), qi[:])
            rt = fpool.tile([P, nwin], f32, tag="rt", name=f"rt{s}")
            for _ in range(3):
                nc.vector.tensor_mul(rt[:], rstd[:], rstd[:])
                nc.vector.tensor_mul(rt[:], rt[:], vr[:])
                nc.vector.tensor_scalar(rt[:], rt[:], -0.5, 1.5,
                                        OP.mult, OP.add)
                nc.vector.tensor_mul(rstd[:], rstd[:], rt[:])
            yn16 = fpool.tile([P, nwin, HID], f16, tag="yn", name=f"yn{s}")
            for wl in range(nwin):
                nc.vector.scalar_tensor_tensor(
                    yn16[:, wl, :], y[:, wl, :], bn_sl[:, wl, 0:1],
                    rstd[:, wl:wl + 1].broadcast_to([P, HID]),
                    OP.subtract, OP.mult)
                if not sched["skip_norm"]:
                    nc.vector.tensor_mul(yn16[:, wl, :], yn16[:, wl, :],
                                         w_rep[:])
                    nc.vector.tensor_add(yn16[:, wl, :], yn16[:, wl, :],
                                         b_rep[:])
            nfull = sum(1 for w in ws if npc - w * P >= P)
            if nfull:
                nc.sync.dma_start(
                    out[ws[0] * P:ws[0] * P + nfull * P, :]
                    .rearrange("(w p) h -> p w h", p=P),
                    yn16[:, :nfull, :])
            for wl, w in enumerate(ws):
                rows = npc - w * P
                if rows < P:
                    nc.sync.dma_start(out[w * P:w * P + rows, :],
                                      yn16[:rows, wl, :])

        nslab = len(sched["slabs"])
        for s in range(min(g.lookahead, nslab)):
            emit_front(s)
        for s in range(nslab):
            if s + g.lookahead < nslab:
                emit_front(s + g.lookahead)
            emit_back(s)
            # flush of the previous slab goes AFTER this slab's back: its
            # first op waits on slab s-1's scatter, and emitting it earlier
            # would park the in-order Act/DVE queues ahead of slab s's work
            if s >= 1:
                emit_flush(s - 1)
        for fn in pending_scat.pop(nslab - 1):
            fn()
        emit_flush(nslab - 1)

    nc.compile()
    return nc


# ---------------------------------------------------------------------------
# Full-input entry point: shard, compile (cached), run SPMD on 8 cores,
# gather the output shards.
# ---------------------------------------------------------------------------
_CACHE = {}


def kernel(**inputs):
    N = int(np.asarray(inputs["node_embeddings"]).shape[0])
    n_cores = 8
    g = Geo(N=N, n_cores=n_cores)

    sched, in_maps = host_prep(g, **{k: np.asarray(v) for k, v in inputs.items()})

    key = (N, sched["total_chunks"], tuple(int(x) for x in sched["caps"].ravel()),
           sched["skip_norm"])
    if key not in _CACHE:
        _CACHE[key] = build_program(g, sched)
    nc = _CACHE[key]

    from concourse.bass_utils import run_bass_kernel_spmd
    res = run_bass_kernel_spmd(nc, in_maps, core_ids=list(range(n_cores)))
    # un-permute the window-balanced rows (device row pos[i] holds node i)
    out = np.concatenate([res.results[c]["out"][sched["poss"][c]]
                          for c in range(n_cores)], axis=0)
    return out.astype(np.float32)
